# revision 1
# baseline (speedup 1.0000x reference)
"""DenseCapsLayer Trainium2 kernel.

Math (per (n, a) pair; A=32 input capsule types, B=32 output, P=4, hw=256):
  votes v[h,b] = W[a,b] @ M[h]  (4x4 matmuls) -- NEVER materialized (256MB).
  Routing reduces to small per-pair contractions:
    Mbar[b]   = sum_h c[h,b] * M[h]          (c = softmax over h of L)
    S[b]      = W[a,b] @ Mbar[b]
    n2[b]     = |S[b]|^2 = <Mbar[b], G[a,b] @ Mbar[b]>,  G = W^T W  (host-precomputed)
    Pout[b]   = f(n2) * S[b]                  (squash factor f)
    U[b]      = W^T Pout[b] = f * G @ Mbar[b]
    L        += M @ U^T  (so L_t = M @ Ubar_t^T with Ubar = cumulative sum of U)
  Final output = Pout at iter 2.

Sharding: data-parallel over batch: core c handles n in {2c, 2c+1} (NL=2), all
32 a's. Per-core layout: 16 "groups" g = j*2 + nl (j = a-block of 4, nl =
local n); partitions = (aL, b) = aL*32 + b with aL = a - 4j.
"""

import numpy as np
import ml_dtypes

import concourse.bass as bass
import concourse.bacc as bacc
import concourse.mybir as mybir
import concourse.tile as tile
from concourse.bass_utils import run_bass_kernel_spmd

F32 = mybir.dt.float32
F16 = mybir.dt.float16
BF16 = mybir.dt.bfloat16

A, B, P, ITERS = 32, 32, 4, 3
PS = P * P                      # 16
BATCH, OH, OW = 16, 16, 16
HW = OH * OW                    # 256
NCORES = 8
NL = BATCH // NCORES            # 2 local batch items per core
J = A // 4                      # 8 groups of 4 a's
G = J * NL                      # 16 (g = j*NL + nl)
NB = 4                          # g-batches for L/exp processing (4 g each)
EPS = 1e-8

AF = mybir.ActivationFunctionType
ALU = mybir.AluOpType
AX = mybir.AxisListType


# ---------------------------------------------------------------- device code
import os as _os
_STOP = _os.environ.get("K_STOP", "")


def _emit(tc, xs16t, xh16, xl16, wga, wws, o32):
    nc = tc.nc

    dbg_view = o32.rearrange("n a b k -> (n a b k)") \
                  .rearrange("(p f) -> p f", f=256)

    def dump(src):
        # debug: copy a (128, 256) fp32 AP to the output
        nc.sync.dma_start(out=dbg_view, in_=src)

    with (
        tc.tile_pool(name="inp", bufs=1) as inp,
        tc.tile_pool(name="state", bufs=1) as state,
        tc.tile_pool(name="work", bufs=3) as work,
        tc.tile_pool(name="small", bufs=2) as small,
        tc.tile_pool(name="lps", bufs=2, space="PSUM") as lps_pool,
        tc.tile_pool(name="mbps", bufs=1, space="PSUM") as mbps_pool,
        tc.tile_pool(name="dram", bufs=2, space="DRAM") as dram,
    ):
        # ---------------- persistent inputs in SBUF (batched DMAs)
        Xh = {}
        Xl = {}
        for ch in range(2):
            th = inp.tile([128, NL * A * PS], BF16, tag=f"xh{ch}")
            nc.sync.dma_start(
                out=th[:].rearrange("p (n c) -> p n c", n=NL),
                in_=xh16[:, ch * 128:(ch + 1) * 128, :].rearrange(
                    "n p c -> p n c"))
            tl = inp.tile([128, NL * A * PS], BF16, tag=f"xl{ch}")
            nc.sync.dma_start(
                out=tl[:].rearrange("p (n c) -> p n c", n=NL),
                in_=xl16[:, ch * 128:(ch + 1) * 128, :].rearrange(
                    "n p c -> p n c"))
            for nl in range(NL):
                Xh[nl, ch] = th[:, nl * A * PS:(nl + 1) * A * PS]
                Xl[nl, ch] = tl[:, nl * A * PS:(nl + 1) * A * PS]

        GA = inp.tile([128, G * 64], F16, tag="ga")
        nc.scalar.dma_start(out=GA[:], in_=wga[:, :])
        WS = inp.tile([128, G * 64], F32, tag="ws")
        nc.scalar.dma_start(out=WS[:], in_=wws[:, :])

        # MTall: (kq, g*1024 + aL*256 + h) fp16 -- M^T pre-transposed on the
        # HOST (static input), one DMA.  All matmul operands must live at
        # partition base 0 in this environment (mixing PE row-groups faults).
        MTall = inp.tile([PS, G * 4 * HW], F16, tag="mtall")
        nc.sync.dma_start(
            out=MTall[:].rearrange("p (g c) -> p g c", g=G),
            in_=xs16t.rearrange("g p c -> p g c"))
        MT16 = {g: MTall[:, g * 4 * HW:(g + 1) * 4 * HW] for g in range(G)}

        ones_bf = inp.tile([128, 128], BF16, tag="ones_bf")
        nc.gpsimd.memset(ones_bf[:], 1.0)
        onecol = inp.tile([128, 1], BF16, tag="onecol")
        nc.gpsimd.memset(onecol[:], 1.0)
        epsc = inp.tile([128, 1], F32, tag="epsc")
        nc.gpsimd.memset(epsc[:], EPS)

        # Preload the combined exp+ln activation table set once; otherwise the
        # table-load pass alternates exp_and_others / natural_log every iter
        # (~1.3us per reload).
        from concourse.hw_specs import get_activation_tables
        _tables = list(get_activation_tables(nc.m.arch).items())
        _set_id = next(i for i, (nm, fns) in enumerate(_tables)
                       if AF.Exp in fns and AF.Ln in fns)
        nc.scalar.add_instruction(mybir.InstLoadActFuncSet(
            name=nc.get_next_instruction_name(),
            ins=[], outs=[], act_func_set_id=_set_id))

        if _STOP == "setup":
            dump(WS[:, 0:256])
            return

        ubar_prev = None
        lps_tiles = {}

        for t in range(ITERS):
            # -------- Mb matmuls (+ exp for t>0), processed in 4-g batches
            mb_ps0 = mbps_pool.tile([128, 8 * 64], F32, tag="mb0")
            mb_ps1 = mbps_pool.tile([128, 8 * 64], F32, tag="mb1")
            mb_ps = [mb_ps0, mb_ps1]
            den_ps = None
            if t > 0:
                den_ps0 = mbps_pool.tile([128, 8 * 2], F32, tag="den0")
                den_ps1 = mbps_pool.tile([128, 8 * 2], F32, tag="den1")
                den_ps = [den_ps0, den_ps1]
            for bi in range(NB):
                el = None
                if t > 0:
                    el = work.tile([128, 1024], BF16, tag="expl")
                    nc.scalar.activation(el[:], lps_tiles[bi][:], AF.Exp)
                for gi in range(4):
                    g = bi * 4 + gi
                    nl, j = g // J, g % J
                    out_g = mb_ps[g // 8][:, (g % 8) * 64:
                                          (g % 8) * 64 + 64]
                    for ch in range(2):
                        if t == 0:
                            lhsT = ones_bf[:]
                        else:
                            lhsT = el[:, gi * 256 + ch * 128:
                                      gi * 256 + (ch + 1) * 128]
                        if t > 0:
                            # denominator first so recd is ready by extraction
                            nc.tensor.matmul(
                                den_ps[g // 8][:, (g % 8) * 2 + ch:
                                               (g % 8) * 2 + ch + 1],
                                lhsT, onecol[:], start=True, stop=True)
                        rx = Xh[nl, ch][:].rearrange(
                            "p (a kq) -> p a kq", kq=PS)[:, 4 * j:4 * j + 4, :]
                        nc.tensor.matmul(out_g, lhsT, rx,
                                         start=(ch == 0), stop=False)
                        rxl = Xl[nl, ch][:].rearrange(
                            "p (a kq) -> p a kq",
                            kq=PS)[:, 4 * j:4 * j + 4, :]
                        nc.tensor.matmul(out_g, lhsT, rxl,
                                         start=False, stop=(ch == 1))

            # ================ post-Mb phase, pipelined per half H
            # (half H = g in [H*8, H*8+8) = local batch item nl == H, cols
            # [H*128, (H+1)*128) of all (g,kq)-shaped tensors)
            ub_halves = {}

            for H in range(2):
                gsl = slice(0, 8)
                csl = slice(0, 128)
                mbv = mb_ps[H][:].rearrange("p (g c) -> p g c", c=64)
                if t < 2:
                    mbar = state.tile([128, 8 * PS], F16, tag=f"mbar{t}{H}")
                    z = state.tile([128, 8 * PS], F32, tag=f"z{t}{H}")
                    ub = state.tile([128, 8 * PS], F16, tag=f"ubar{t}{H}")
                    uta = work.tile([PS, 8 * 128], F16, tag=f"uta{H}")
                else:
                    mbar = state.tile([128, 8 * PS], F32, tag=f"mbar32{H}")
                    s = state.tile([128, 8 * PS], F32, tag=f"s{H}")
                    outsb = state.tile([128, 8 * PS], F32, tag=f"outsb{H}")
                mview = mbar[:].rearrange("p (g kq) -> p g kq", kq=PS)

                # ---- denominators for this half
                recd = None
                if t > 0:
                    dview = den_ps[H][:].rearrange("p (g c) -> p g c",
                                                   c=2)
                    dcp = small.tile([128, 8], F32, tag=f"dcp{H}")
                    nc.vector.tensor_copy(dcp[:], dview[:, gsl, 1])
                    dsum = small.tile([128, 8], F32, tag=f"dsum{H}")
                    nc.vector.tensor_add(dsum[:], dview[:, gsl, 0], dcp[:])
                    recd = small.tile([128, 8], F32, tag=f"recd{H}")
                    nc.vector.reciprocal(recd[:], dsum[:])

                # ---- extract diagonal blocks + normalize
                for aL in range(4):
                    src_ = mbv[aL * 32:(aL + 1) * 32, gsl,
                               aL * 16:aL * 16 + 16]
                    dst_ = mview[aL * 32:(aL + 1) * 32]
                    if t == 0:
                        if aL < 2:
                            nc.vector.tensor_scalar_mul(dst_, src_, 1.0 / HW)
                        else:
                            nc.scalar.activation(dst_, src_, AF.Identity,
                                                 scale=1.0 / HW)
                    else:
                        rb = recd[aL * 32:(aL + 1) * 32].unsqueeze(2) \
                            .broadcast_to((32, 8, PS))
                        nc.vector.tensor_tensor(dst_, src_, rb, op=ALU.mult)

                if t < 2:
                    # ---- Z = G @ Mbar (fp16 elementwise + add tree)
                    tz = work.tile([128, 8 * 64], F16, tag=f"tz{H}")
                    tzv = tz[:].rearrange("p (g kp k q) -> p g kp k q",
                                          kp=4, k=4, q=4)
                    gav = GA[:].rearrange("p (g kp k q) -> p g kp k q",
                                          kp=4, k=4, q=4)[:, gsl]
                    min1 = mview.rearrange(
                        "p g (kp q) -> p g kp q", q=4) \
                        .unsqueeze(3).broadcast_to((128, 8, 4, 4, 4))
                    nc.vector.tensor_tensor(tzv, gav, min1, op=ALU.mult)
                    tzs = tz[:].rearrange("p (g kp k q) -> p kp g k q",
                                          kp=4, k=4, q=4)
                    t01 = work.tile([128, 8 * PS], F16, tag=f"t01{H}")
                    t01v = t01[:].rearrange("p (g k q) -> p g k q", k=4, q=4)
                    nc.vector.tensor_add(t01v, tzs[:, 0], tzs[:, 1])
                    t23 = work.tile([128, 8 * PS], F16, tag=f"t23{H}")
                    t23v = t23[:].rearrange("p (g k q) -> p g k q", k=4, q=4)
                    nc.vector.tensor_add(t23v, tzs[:, 2], tzs[:, 3])
                    nc.vector.tensor_add(z[:], t01[:], t23[:])
                    # ---- n2 = <Mbar, Z>
                    mz = state.tile([128, 8 * PS], F32, tag=f"mz{H}")
                    nc.vector.tensor_mul(mz[:], mbar[:], z[:])
                    n2 = small.tile([128, 8], F32, tag=f"n2{H}")
                    nc.vector.tensor_reduce(
                        out=n2[:],
                        in_=mz[:].rearrange("p (g kq) -> p g kq", kq=PS),
                        op=ALU.add, axis=AX.X)
                else:
                    # ---- final S = W @ Mbar (fp32 elementwise path)
                    ts = work.tile([128, 8 * 64], F32, tag=f"ts{H}")
                    tsv = ts[:].rearrange("p (g k pp q) -> p g k pp q",
                                          k=4, pp=4, q=4)
                    wsv = WS[:].rearrange("p (g k pp q) -> p g k pp q",
                                          k=4, pp=4, q=4)[:, gsl]
                    min2 = mview.rearrange(
                        "p g (k q) -> p g k q", q=4) \
                        .unsqueeze(3).broadcast_to((128, 8, 4, 4, 4))
                    nc.vector.tensor_tensor(tsv, wsv, min2, op=ALU.mult)
                    nc.vector.tensor_reduce(
                        out=s[:].rearrange("p (g pq) -> p g pq", pq=PS),
                        in_=ts[:].rearrange("p (g k pp q) -> p g pp q k",
                                            k=4, pp=4, q=4),
                        op=ALU.add, axis=AX.X)
                    mz = state.tile([128, 8 * PS], F32, tag=f"mz{H}")
                    nc.vector.tensor_mul(mz[:], s[:], s[:])
                    n2 = small.tile([128, 8], F32, tag=f"n2{H}")
                    nc.vector.tensor_reduce(
                        out=n2[:],
                        in_=mz[:].rearrange("p (g kq) -> p g kq", kq=PS),
                        op=ALU.add, axis=AX.X)

                # ---- squash factor f = n2/(1+n2)/sqrt(n2+eps)
                tln = small.tile([128, 8], F32, tag=f"tln{H}")
                nc.scalar.activation(tln[:], n2[:], AF.Ln, bias=epsc[:])
                rr = small.tile([128, 8], F32, tag=f"rr{H}")
                nc.scalar.activation(rr[:], tln[:], AF.Exp, scale=-0.5)
                dd = small.tile([128, 8], F32, tag=f"dd{H}")
                nc.vector.tensor_scalar_add(dd[:], n2[:], 1.0)
                rec = small.tile([128, 8], F32, tag=f"rec{H}")
                nc.vector.reciprocal(rec[:], dd[:])
                ff = small.tile([128, 8], F32, tag=f"ff{H}")
                nc.vector.tensor_mul(ff[:], n2[:], rec[:])
                ff2 = small.tile([128, 8], F32, tag=f"ff2{H}")
                nc.vector.tensor_mul(ff2[:], ff[:], rr[:])
                fbc = ff2[:].unsqueeze(2).broadcast_to((128, 8, PS))

                if t == 2:
                    # ---- output Pout = f * S; half H is local batch item H
                    nc.vector.tensor_tensor(
                        outsb[:].rearrange("p (g kq) -> p g kq", kq=PS),
                        s[:].rearrange("p (g kq) -> p g kq", kq=PS),
                        fbc, op=ALU.mult)
                    src_o = outsb[:].rearrange("p (jj kq) -> p jj kq",
                                               kq=PS)
                    dst_o = o32[H].rearrange("(jj aL) b kq -> (aL b) jj kq",
                                             jj=J)
                    nc.sync.dma_start(out=dst_o, in_=src_o)
                    continue

                # ---- U = f*Z ; Ubar += U
                ubv = ub[:].rearrange("p (g kq) -> p g kq", kq=PS)
                zv = z[:].rearrange("p (g kq) -> p g kq", kq=PS)
                if t == 0:
                    nc.vector.tensor_tensor(ubv, zv, fbc, op=ALU.mult)
                else:
                    u16 = state.tile([128, 8 * PS], F16, tag=f"u16{H}")
                    nc.vector.tensor_tensor(
                        u16[:].rearrange("p (g kq) -> p g kq", kq=PS),
                        zv, fbc, op=ALU.mult)
                    nc.vector.tensor_add(ub[:], ubar_prev[H][:],
                                         u16[:])

                # ---- UT: xbar transpose + DRAM round-trip to partition 0
                ub_halves[H] = ub
                qeng = nc.sync
                uth = work.tile([128, 128], F16, tag=f"uth{H}")
                qeng.dma_start_transpose(out=uth[:], in_=ub[:])
                udr = dram.tile([128, 128], F16, tag=f"udr{H}")
                qeng.dma_start(out=udr[:], in_=uth[:])
                qeng.dma_start(
                    out=uta[:].rearrange("p (gl ab) -> p gl ab", gl=8),
                    in_=udr[:].rearrange("(gl kq) ab -> kq gl ab", kq=16))
                ut16 = {g: uta[:, (g - H * 8) * 128:(g - H * 8 + 1) * 128]
                        for g in range(H * 8, H * 8 + 8)}

                # ---- L matmuls for next iter (this half's groups)
                for bi in (H * 2, H * 2 + 1):
                    lp = lps_pool.tile([128, 1024], F32, tag="lps")
                    lps_tiles[bi] = lp
                    for gi in range(4):
                        g = bi * 4 + gi
                        for ch in range(2):
                            for aL in range(4):
                                lhsT = MT16[g][0:PS,
                                               aL * 256 + ch * 128:
                                               aL * 256 + (ch + 1) * 128]
                                rhs = ut16[g][0:PS, aL * 32:(aL + 1) * 32]
                                nc.tensor.matmul(
                                    lp[:, gi * 256 + ch * 128 + aL * 32:
                                       gi * 256 + ch * 128 + (aL + 1) * 32],
                                    lhsT, rhs, start=True, stop=True)
            if t < 2:
                ubar_prev = ub_halves
            if _STOP == f"t{t}l":
                dmp = state.tile([128, 256], F32, tag="dmp")
                nc.vector.tensor_copy(dmp[:], lps_tiles[0][:, 0:256])
                dump(dmp[:])
                return


def _build_kernel():
    nc = bacc.Bacc("TRN2", target_bir_lowering=False, debug=False,
                   num_devices=NCORES)
    xs16t = nc.dram_tensor("xs16t", [G, PS, 4 * HW], F16,
                           kind="ExternalInput").ap()
    xh16 = nc.dram_tensor("xh16", [NL, HW, A * PS], BF16,
                          kind="ExternalInput").ap()
    xl16 = nc.dram_tensor("xl16", [NL, HW, A * PS], BF16,
                          kind="ExternalInput").ap()
    wga = nc.dram_tensor("wga", [128, G * 64], F16, kind="ExternalInput").ap()
    wws = nc.dram_tensor("wws", [128, G * 64], F32, kind="ExternalInput").ap()
    o32 = nc.dram_tensor("o32", [NL, A, B, PS], F32,
                         kind="ExternalOutput").ap()

    with tile.TileContext(nc) as tc:
        _emit(tc, xs16t, xh16, xl16, wga, wws, o32)

    nc.compile()
    return nc


# ---------------------------------------------------------------- host side
def _host_weights(weights):
    W = np.asarray(weights, np.float32)                # (A, B, P, P)
    Gm = np.einsum("abpk,abpl->abkl", W, W)            # (A, B, 4, 4): G[k, kp]
    Gsw = np.swapaxes(Gm, 2, 3)                        # Gsw[a,b,kp,k]=Gm[k,kp]
    Wsw = np.swapaxes(W, 2, 3)                         # Wsw[a,b,k,pp]=W[pp,k]

    wga = np.zeros((4, B, G, 4, 4, 4), np.float32)     # (aL,b,g,kp,k,q)
    wws = np.zeros((4, B, G, 4, 4, 4), np.float32)     # (aL,b,g,k,pp,q)
    for g in range(G):
        j = g % J                                      # g = nl*8 + j
        wga[:, :, g] = Gsw[4 * j:4 * j + 4, :, :, :, None]
        wws[:, :, g] = Wsw[4 * j:4 * j + 4, :, :, :, None]
    wga = wga.reshape(4 * B, G * 64)
    wws = wws.reshape(4 * B, G * 64)
    return wga.astype(np.float16), wws.astype(np.float32)


def _host_prep(x, weights):
    xr = np.asarray(x, np.float32).reshape(BATCH, HW, A, PS)
    wga, wws = _host_weights(weights)

    in_maps = []
    for c in range(NCORES):
        xc = xr[c * NL:(c + 1) * NL]                   # (NL, HW, A, PS)
        xh = xc.astype(ml_dtypes.bfloat16)
        xl = (xc - xh.astype(np.float32)).astype(ml_dtypes.bfloat16)
        # xs16t[g, kq, aL*256 + h] = x[nl, h, 4j+aL, kq];  g = nl*8 + j
        xj = xc.reshape(NL, HW, J, 4, PS)              # (nl,h,j,aL,kq)
        xs16t = xj.transpose(0, 2, 4, 3, 1).astype(np.float16)  # nl,j,kq,aL,h
        in_maps.append({
            "xs16t": np.ascontiguousarray(xs16t.reshape(G, PS, 4 * HW)),
            "xh16": np.ascontiguousarray(xh.reshape(NL, HW, A * PS)),
            "xl16": np.ascontiguousarray(xl.reshape(NL, HW, A * PS)),
            "wga": wga,
            "wws": wws,
        })
    return in_maps


_NC_CACHE = {}


def kernel(x, weights):
    if "nc" not in _NC_CACHE:
        _NC_CACHE["nc"] = _build_kernel()
    nc = _NC_CACHE["nc"]
    in_maps = _host_prep(x, weights)
    res = run_bass_kernel_spmd(nc, in_maps, list(range(NCORES)))
    out = np.concatenate([res.results[c]["o32"] for c in range(NCORES)],
                         axis=0)
    return out.astype(np.float32)



# revision 27
# speedup vs baseline: 1.0441x; 1.0441x over previous
"""DenseCapsLayer Trainium2 kernel (v2 — no DRAM round-trip).

Math (per (n, a) pair; A=32 input capsule types, B=32 output, P=4, hw=256):
  votes v[h,b] = W[a,b] @ M[h]  (4x4 matmuls) -- NEVER materialized (256MB).
  Routing reduces to small per-pair contractions:
    Mbar[b]   = sum_h c[h,b] * M[h]          (c = softmax over h of L)
    S[b]      = W[a,b] @ Mbar[b]
    n2[b]     = |S[b]|^2 = <Mbar[b], G[a,b] @ Mbar[b]>,  G = W^T W  (host-precomputed)
    Pout[b]   = f(n2) * S[b]                  (squash factor f)
    U[b]      = W^T Pout[b] = f * G @ Mbar[b]
    L        += M @ U^T  (so L_t = M @ Ubar_t^T with Ubar = cumulative sum of U)
  Final output = Pout at iter 2.

Sharding: data-parallel over batch: core c handles n in {2c, 2c+1} (NL=2), all
32 a's.  Groups g = nl*8 + j (j = a-block of 4); partitions = (aL, b).

v2 key changes vs v1:
  * U^T is produced with a PE transpose (matmul vs identity) + 16 small
    stripe copies into a pre-zeroed block-sparse SBUF tile UTS
    [(gL kq), (aL, gL, b)] -- the v1 SBUF->DRAM->SBUF partition-regroup
    round trip (~7us/iter of serialized DMA latency) is gone.
  * L matmuls contract over (gL kq)=128 with a static dense M^T operand
    (MTD); cross-group terms vanish against UTS's static zeros.
  * Both batch-halves are processed in single wide DVE ops (half the
    instruction count on the serial DVE dependency chain).
  * exp(L) is emitted right after each lp tile's matmuls (tail of the
    same iteration) so it runs off the critical path.
  * wga/wws shrunk 4x (q-broadcast moved on-device via stride-0 views).
"""

import numpy as np
import ml_dtypes

import concourse.bass as bass
import concourse.bacc as bacc
import concourse.mybir as mybir
import concourse.tile as tile
from concourse.bass_utils import run_bass_kernel_spmd

F32 = mybir.dt.float32
F16 = mybir.dt.float16
BF16 = mybir.dt.bfloat16

A, B, P, ITERS = 32, 32, 4, 3
PS = P * P                      # 16
BATCH, OH, OW = 16, 16, 16
HW = OH * OW                    # 256
NCORES = 8
NL = BATCH // NCORES            # 2 local batch items per core
J = A // 4                      # 8 j-blocks of 4 a's
G = J * NL                      # 16 groups (g = nl*8 + j)
EPS = 1e-8

AF = mybir.ActivationFunctionType
ALU = mybir.AluOpType
AX = mybir.AxisListType


# ---------------------------------------------------------------- device code
import os as _os
_STOP = _os.environ.get("K_STOP", "")


def _emit(tc, mtd, xh16, xl16, wga, wws, iden, o32):
    nc = tc.nc

    dbg_view = o32.rearrange("n a b k -> (n a b k)") \
                  .rearrange("(p f) -> p f", f=256)

    def dump(src):
        # debug: copy a (128, 256) fp32 AP to the output
        nc.sync.dma_start(out=dbg_view, in_=src)

    with (
        tc.tile_pool(name="inp", bufs=1) as inp,
        tc.tile_pool(name="state", bufs=1) as state,
        tc.tile_pool(name="work", bufs=3) as work,
        tc.tile_pool(name="small", bufs=2) as small,
        tc.tile_pool(name="elp", bufs=4) as elp,
        tc.tile_pool(name="lps", bufs=2, space="PSUM") as lps_pool,
        tc.tile_pool(name="mbps", bufs=1, space="PSUM") as mbps_pool,
    ):
        # ---------------- constants + zeroed UTS first (Pool engine, no deps)
        ones_bf = inp.tile([128, 128], BF16, tag="ones_bf")
        nc.gpsimd.memset(ones_bf[:], 1.0)
        onecol = inp.tile([128, 1], BF16, tag="onecol")
        nc.gpsimd.memset(onecol[:], 1.0)
        epsc = inp.tile([128, 1], F32, tag="epsc")
        nc.gpsimd.memset(epsc[:], EPS)
        # UTS: [(g4 kq32), (bi, aL, g4, b)] block-sparse U^T with kq padded to
        # 32 so every stripe is 32-partition aligned (engine SBUF access
        # granularity).  Zeros are static; only stripe (g4) of each column
        # block is rewritten each iteration.  Junk in kq>=16 rows is killed
        # by MTD32's static zero rows.
        UTS = inp.tile([128, 4 * 4 * 4 * 32], F16, tag="uts")
        nc.gpsimd.memset(UTS[:], 0.0)
        # ping-pong Ubar tiles with kq padded to 32; pad columns are zeroed
        # once here and never written again.
        UB = []
        for i in range(2):
            u = inp.tile([128, G * 32], F16, tag=f"ub{i}")
            nc.gpsimd.memset(u[:], 0.0)
            UB.append(u)

        # ---------------- persistent inputs in SBUF
        # x (h/l bf16 split) on the sync queue -- needed first.
        Xh = {}
        Xl = {}
        for ch in range(2):
            th = inp.tile([128, NL * A * PS], BF16, tag=f"xh{ch}")
            nc.sync.dma_start(
                out=th[:].rearrange("p (n c) -> p n c", n=NL),
                in_=xh16[:, ch * 128:(ch + 1) * 128, :].rearrange(
                    "n p c -> p n c"))
            tl = inp.tile([128, NL * A * PS], BF16, tag=f"xl{ch}")
            nc.sync.dma_start(
                out=tl[:].rearrange("p (n c) -> p n c", n=NL),
                in_=xl16[:, ch * 128:(ch + 1) * 128, :].rearrange(
                    "n p c -> p n c"))
            for nl in range(NL):
                Xh[nl, ch] = th[:, nl * A * PS:(nl + 1) * A * PS]
                Xl[nl, ch] = tl[:, nl * A * PS:(nl + 1) * A * PS]

        # weights (vector queue) + M^T/identity (scalar queue): needed later,
        # loaded concurrently with the t=0 Mb matmuls.
        GA = inp.tile([128, G * 16], F16, tag="ga")
        nc.gpsimd.dma_start(out=GA[:], in_=wga[:, :])
        WS = inp.tile([128, G * 16], F32, tag="ws")
        nc.gpsimd.dma_start(out=WS[:], in_=wws[:, :])
        IDEN = inp.tile([128, 128], F16, tag="iden")
        nc.scalar.dma_start(out=IDEN[:], in_=iden[:, :])
        # MTD32[(g4 kq32), ((bi, ch, aL), h)] = M_{g=bi*4+g4}[ch*128+h, aL, kq]
        # for kq<16; rows kq>=16 are zero (host-prepared; plain full-tile DMA
        # -- partition-subset DMA views mis-lower to flat descriptors).
        MTD32 = inp.tile([128, 32 * 128], F16, tag="mtd32")
        nc.scalar.dma_start(out=MTD32[:], in_=mtd[:, :])

        # Preload the combined exp+ln activation table set once.
        from concourse.hw_specs import get_activation_tables
        _tables = list(get_activation_tables(nc.m.arch).items())
        _set_id = next(i for i, (nm, fns) in enumerate(_tables)
                       if AF.Exp in fns and AF.Ln in fns)
        nc.scalar.add_instruction(mybir.InstLoadActFuncSet(
            name=nc.get_next_instruction_name(),
            ins=[], outs=[], act_func_set_id=_set_id))

        el_tiles = {}

        for t in range(ITERS):
            # ---------------- Mb matmuls (all 16 groups, one PSUM tile)
            mb_ps = mbps_pool.tile([128, G * 64], F32, tag="mb")
            den_ps = None
            if t > 0:
                den_ps = mbps_pool.tile([128, G * 2], F32, tag="den")
            for g in range(G):
                nl, j = g // J, g % J
                bi, gi = g // 4, g % 4
                out_g = mb_ps[:, g * 64:(g + 1) * 64]
                for ch in range(2):
                    if t == 0:
                        lhsT = ones_bf[:]
                    else:
                        lhsT = el_tiles[bi][:, ch * 512 + gi * 128:
                                            ch * 512 + (gi + 1) * 128]
                        nc.tensor.matmul(
                            den_ps[:, g * 2 + ch:g * 2 + ch + 1],
                            lhsT, onecol[:], start=True, stop=True)
                    rx = Xh[nl, ch][:].rearrange(
                        "p (a kq) -> p a kq", kq=PS)[:, 4 * j:4 * j + 4, :]
                    nc.tensor.matmul(out_g, lhsT, rx,
                                     start=(ch == 0), stop=False)
                    rxl = Xl[nl, ch][:].rearrange(
                        "p (a kq) -> p a kq", kq=PS)[:, 4 * j:4 * j + 4, :]
                    nc.tensor.matmul(out_g, lhsT, rxl,
                                     start=False, stop=(ch == 1))

            def _dump32(src_ap, n=256):
                dmp = state.tile([128, 256], F32, tag="dmp")
                nc.gpsimd.memset(dmp[:], 0.0)
                nc.vector.tensor_copy(dmp[:, 0:n], src_ap)
                dump(dmp[:])

            if _STOP == f"t{t}mb":
                _dump32(mb_ps[:, 0:256])
                return

            # ---------------- extract diagonal blocks + normalize (merged)
            mbv = mb_ps[:].rearrange("p (g c) -> p g c", c=64)
            if t < 2:
                mbar = state.tile([128, G * PS], F16, tag=f"mbar{t}")
                z = state.tile([128, G * PS], F32, tag=f"z{t}")
            else:
                mbar = state.tile([128, G * PS], F32, tag="mbar32")
                s = state.tile([128, G * PS], F32, tag="sfin")
                outsb = state.tile([128, G * PS], F32, tag="outsb")
            mview = mbar[:].rearrange("p (g kq) -> p g kq", kq=PS)

            recd = None
            if t > 0:
                dview = den_ps[:].rearrange("p (g c) -> p g c", c=2)
                dcp = small.tile([128, G], F32, tag="dcp")
                nc.vector.tensor_copy(dcp[:], dview[:, :, 1])
                dsum = small.tile([128, G], F32, tag="dsum")
                nc.vector.tensor_add(dsum[:], dview[:, :, 0], dcp[:])
                recd = small.tile([128, G], F32, tag="recd")
                nc.vector.reciprocal(recd[:], dsum[:])

            for aL in range(4):
                src_ = mbv[aL * 32:(aL + 1) * 32, :, aL * 16:aL * 16 + 16]
                dst_ = mview[aL * 32:(aL + 1) * 32]
                if t == 0:
                    if aL < 2:
                        nc.vector.tensor_scalar_mul(dst_, src_, 1.0 / HW)
                    else:
                        nc.scalar.activation(dst_, src_, AF.Identity,
                                             scale=1.0 / HW)
                else:
                    rb = recd[aL * 32:(aL + 1) * 32].unsqueeze(2) \
                        .broadcast_to((32, G, PS))
                    nc.vector.tensor_tensor(dst_, src_, rb, op=ALU.mult)

            if _STOP == f"t{t}mbar":
                _dump32(mbar[:, 0:256])
                return

            if t < 2:
                # ---- Z = G @ Mbar (f16 elementwise + add tree)
                tz = work.tile([128, G * 64], F16, tag="tz")
                tzv = tz[:].rearrange("p (g kp k q) -> p g kp k q",
                                      kp=4, k=4, q=4)
                gav = GA[:].rearrange("p (g kp k) -> p g kp k",
                                      kp=4, k=4).unsqueeze(4) \
                    .broadcast_to((128, G, 4, 4, 4))
                min1 = mview.rearrange(
                    "p g (kp q) -> p g kp q", q=4) \
                    .unsqueeze(3).broadcast_to((128, G, 4, 4, 4))
                nc.vector.tensor_tensor(tzv, gav, min1, op=ALU.mult)
                tzs = tz[:].rearrange("p (g kp k q) -> p kp g k q",
                                      kp=4, k=4, q=4)
                t01 = work.tile([128, G * PS], F16, tag="t01")
                t01v = t01[:].rearrange("p (g k q) -> p g k q", k=4, q=4)
                nc.vector.tensor_add(t01v, tzs[:, 0], tzs[:, 1])
                t23 = work.tile([128, G * PS], F16, tag="t23")
                t23v = t23[:].rearrange("p (g k q) -> p g k q", k=4, q=4)
                nc.vector.tensor_add(t23v, tzs[:, 2], tzs[:, 3])
                nc.vector.tensor_add(z[:], t01[:], t23[:])
                # ---- n2 = <Mbar, Z>
                mz = work.tile([128, G * PS], F32, tag="mz")
                nc.vector.tensor_mul(mz[:], mbar[:], z[:])
                n2 = small.tile([128, G], F32, tag="n2")
                nc.vector.tensor_reduce(
                    out=n2[:],
                    in_=mz[:].rearrange("p (g kq) -> p g kq", kq=PS),
                    op=ALU.add, axis=AX.X)
            else:
                # ---- final S = W @ Mbar (fp32 elementwise path)
                ts = work.tile([128, G * 64], F32, tag="tsf")
                tsv = ts[:].rearrange("p (g k pp q) -> p g k pp q",
                                      k=4, pp=4, q=4)
                wsv = WS[:].rearrange("p (g k pp) -> p g k pp",
                                      k=4, pp=4).unsqueeze(4) \
                    .broadcast_to((128, G, 4, 4, 4))
                min2 = mview.rearrange(
                    "p g (k q) -> p g k q", q=4) \
                    .unsqueeze(3).broadcast_to((128, G, 4, 4, 4))
                nc.vector.tensor_tensor(tsv, wsv, min2, op=ALU.mult)
                nc.vector.tensor_reduce(
                    out=s[:].rearrange("p (g pq) -> p g pq", pq=PS),
                    in_=ts[:].rearrange("p (g k pp q) -> p g pp q k",
                                        k=4, pp=4, q=4),
                    op=ALU.add, axis=AX.X)
                mz = work.tile([128, G * PS], F32, tag="mz")
                nc.vector.tensor_mul(mz[:], s[:], s[:])
                n2 = small.tile([128, G], F32, tag="n2")
                nc.vector.tensor_reduce(
                    out=n2[:],
                    in_=mz[:].rearrange("p (g kq) -> p g kq", kq=PS),
                    op=ALU.add, axis=AX.X)

            # ---- squash factor f = n2/(1+n2)/sqrt(n2+eps)
            tln = small.tile([128, G], F32, tag="tln")
            nc.scalar.activation(tln[:], n2[:], AF.Ln, bias=epsc[:])
            rr = small.tile([128, G], F32, tag="rr")
            nc.scalar.activation(rr[:], tln[:], AF.Exp, scale=-0.5)
            dd = small.tile([128, G], F32, tag="dd")
            nc.vector.tensor_scalar_add(dd[:], n2[:], 1.0)
            rec = small.tile([128, G], F32, tag="rec")
            nc.vector.reciprocal(rec[:], dd[:])
            ff = small.tile([128, G], F32, tag="ff")
            nc.vector.tensor_mul(ff[:], n2[:], rec[:])
            ff2 = small.tile([128, G], F32, tag="ff2")
            nc.vector.tensor_mul(ff2[:], ff[:], rr[:])
            fbc = ff2[:].unsqueeze(2).broadcast_to((128, G, PS))

            if t == 2:
                # ---- output Pout = f * S
                nc.vector.tensor_tensor(
                    outsb[:].rearrange("p (g kq) -> p g kq", kq=PS),
                    s[:].rearrange("p (g kq) -> p g kq", kq=PS),
                    fbc, op=ALU.mult)
                src_a = outsb[:].rearrange("p (g kq) -> p g kq", kq=PS)
                for H in range(2):
                    dst_o = o32[H].rearrange("(jj aL) b kq -> (aL b) jj kq",
                                             jj=J)
                    nc.sync.dma_start(out=dst_o,
                                      in_=src_a[:, H * 8:(H + 1) * 8, :])
                continue

            # ---- U = f*Z ; Ubar += U   (UB tiles have kq padded to 32)
            ub = UB[t]
            ubv = ub[:].rearrange("p (g kq2) -> p g kq2", kq2=32)[:, :, 0:PS]
            zv = z[:].rearrange("p (g kq) -> p g kq", kq=PS)
            if t == 0:
                nc.vector.tensor_tensor(ubv, zv, fbc, op=ALU.mult)
            else:
                u16 = work.tile([128, G * PS], F16, tag="u16")
                u16v = u16[:].rearrange("p (g kq) -> p g kq", kq=PS)
                nc.vector.tensor_tensor(u16v, zv, fbc, op=ALU.mult)
                ub0v = UB[0][:].rearrange("p (g kq2) -> p g kq2",
                                          kq2=32)[:, :, 0:PS]
                nc.vector.tensor_tensor(ubv, ub0v, u16v, op=ALU.add)

            if _STOP == f"t{t}ub":
                _dump32(ub[:, 0:256])
                return

            # ---- U^T via PE transpose (per 4-group quarter) into PSUM
            uthp = mbps_pool.tile([128, 512], F16, tag="uthp")
            for bi in range(4):
                nc.tensor.transpose(uthp[:, bi * 128:(bi + 1) * 128],
                                    ub[:, bi * 128:(bi + 1) * 128], IDEN[:])
            # ---- 4 aligned stripe copies into block-sparse UTS
            for g4 in range(4):
                src = uthp[g4 * 32:(g4 + 1) * 32, :].rearrange(
                    "p (bi aL b) -> p bi aL b", bi=4, aL=4)
                dst = UTS[g4 * 32:(g4 + 1) * 32].rearrange(
                    "p (bi aL g4c b) -> p bi aL g4c b",
                    bi=4, aL=4, g4c=4)[:, :, :, g4, :]
                if g4 % 2 == 0:
                    nc.scalar.activation(dst, src, AF.Identity)
                else:
                    nc.vector.tensor_copy(dst, src)

            if _STOP == f"t{t}uts":
                _dump32(UTS[:, 0:256])
                return
            if _STOP == f"t{t}uthp":
                _dump32(uthp[:, 0:256])
                return

            # ---- L matmuls: L[h, (aL b)] = sum_(g4 kq32) MTD32 * UTS
            lp_tiles = {}
            for bi in range(4):
                lp = lps_pool.tile([128, 1024], F32, tag="lps")
                lp_tiles[bi] = lp
                for gi in range(4):
                    for ch in range(2):
                        for aL in range(4):
                            lhsT = MTD32[:, ((bi * 2 + ch) * 4 + aL) * 128:
                                         ((bi * 2 + ch) * 4 + aL + 1) * 128]
                            rhs = UTS[:, (bi * 16 + aL * 4 + gi) * 32:
                                      (bi * 16 + aL * 4 + gi + 1) * 32]
                            nc.tensor.matmul(
                                lp[:, ch * 512 + gi * 128 + aL * 32:
                                   ch * 512 + gi * 128 + (aL + 1) * 32],
                                lhsT, rhs, start=True, stop=True)
                # exp of this lp right away (off the critical path)
                el = elp.tile([128, 1024], BF16, tag="el")
                nc.scalar.activation(el[:], lp[:], AF.Exp)
                el_tiles[bi] = el
            if _STOP == f"t{t}lp":
                _dump32(lp_tiles[0][:, 0:256])
                return


def _build_kernel():
    nc = bacc.Bacc("TRN2", target_bir_lowering=False, debug=False,
                   num_devices=NCORES)
    mtd = nc.dram_tensor("mtd", [128, 32 * 128], F16,
                         kind="ExternalInput").ap()
    xh16 = nc.dram_tensor("xh16", [NL, HW, A * PS], BF16,
                          kind="ExternalInput").ap()
    xl16 = nc.dram_tensor("xl16", [NL, HW, A * PS], BF16,
                          kind="ExternalInput").ap()
    wga = nc.dram_tensor("wga", [128, G * 16], F16, kind="ExternalInput").ap()
    wws = nc.dram_tensor("wws", [128, G * 16], F32, kind="ExternalInput").ap()
    iden = nc.dram_tensor("iden", [128, 128], F16, kind="ExternalInput").ap()
    o32 = nc.dram_tensor("o32", [NL, A, B, PS], F32,
                         kind="ExternalOutput").ap()

    with tile.TileContext(nc) as tc:
        _emit(tc, mtd, xh16, xl16, wga, wws, iden, o32)

    nc.compile()
    return nc


# ---------------------------------------------------------------- host side
def _host_weights(weights):
    W = np.asarray(weights, np.float32)                # (A, B, P, P)
    Gm = np.einsum("abpk,abpl->abkl", W, W)            # (A, B, 4, 4)
    Gsw = np.swapaxes(Gm, 2, 3)                        # Gsw[a,b,kp,k]=Gm[k,kp]
    Wsw = np.swapaxes(W, 2, 3)                         # Wsw[a,b,k,pp]=W[pp,k]

    wga = np.zeros((4, B, G, 4, 4), np.float32)        # (aL,b,g,kp,k)
    wws = np.zeros((4, B, G, 4, 4), np.float32)        # (aL,b,g,k,pp)
    for g in range(G):
        j = g % J                                      # g = nl*8 + j
        wga[:, :, g] = Gsw[4 * j:4 * j + 4]
        wws[:, :, g] = Wsw[4 * j:4 * j + 4]
    wga = wga.reshape(4 * B, G * 16)
    wws = wws.reshape(4 * B, G * 16)
    return wga.astype(np.float16), wws.astype(np.float32)


def _host_prep(x, weights):
    xr = np.asarray(x, np.float32).reshape(BATCH, HW, A, PS)
    wga, wws = _host_weights(weights)
    iden = np.eye(128, dtype=np.float16)

    in_maps = []
    for c in range(NCORES):
        xc = xr[c * NL:(c + 1) * NL]                   # (NL, HW, A, PS)
        xh = xc.astype(ml_dtypes.bfloat16)
        xl = (xc - xh.astype(np.float32)).astype(ml_dtypes.bfloat16)
        # mtd[(g4 kq32), ((bi, ch, aL), h)] = xc[nl, ch*128+h, 4j+aL, kq]
        # for kq<16 (zeros at kq>=16), with g = bi*4 + g4 = nl*8 + j.
        xj = xc.reshape(NL, 2, 128, J, 4, PS)          # nl,ch,h,j,aL,kq
        mt = np.zeros((4, 32, 4, 2, 4, 128), np.float32)
        for bi in range(4):
            nl = bi // 2
            for g4 in range(4):
                j = (bi % 2) * 4 + g4
                # xj[nl, :, :, j] is (ch, h, aL, kq) -> (kq, ch, aL, h)
                mt[g4, 0:PS, bi] = xj[nl, :, :, j].transpose(3, 0, 2, 1)
        mtdc = mt.reshape(128, 4096).astype(np.float16)
        in_maps.append({
            "mtd": np.ascontiguousarray(mtdc),
            "xh16": np.ascontiguousarray(xh.reshape(NL, HW, A * PS)),
            "xl16": np.ascontiguousarray(xl.reshape(NL, HW, A * PS)),
            "wga": wga,
            "wws": wws,
            "iden": iden,
        })
    return in_maps


_NC_CACHE = {}


def kernel(x, weights):
    if "nc" not in _NC_CACHE:
        _NC_CACHE["nc"] = _build_kernel()
    nc = _NC_CACHE["nc"]
    in_maps = _host_prep(x, weights)
    res = run_bass_kernel_spmd(nc, in_maps, list(range(NCORES)))
    out = np.concatenate([res.results[c]["o32"] for c in range(NCORES)],
                         axis=0)
    return out.astype(np.float32)


# revision 48
# speedup vs baseline: 1.3671x; 1.3093x over previous
"""DenseCapsLayer Trainium2 kernel (v2 — no DRAM round-trip).

Math (per (n, a) pair; A=32 input capsule types, B=32 output, P=4, hw=256):
  votes v[h,b] = W[a,b] @ M[h]  (4x4 matmuls) -- NEVER materialized (256MB).
  Routing reduces to small per-pair contractions:
    Mbar[b]   = sum_h c[h,b] * M[h]          (c = softmax over h of L)
    S[b]      = W[a,b] @ Mbar[b]
    n2[b]     = |S[b]|^2 = <Mbar[b], G[a,b] @ Mbar[b]>,  G = W^T W  (host-precomputed)
    Pout[b]   = f(n2) * S[b]                  (squash factor f)
    U[b]      = W^T Pout[b] = f * G @ Mbar[b]
    L        += M @ U^T  (so L_t = M @ Ubar_t^T with Ubar = cumulative sum of U)
  Final output = Pout at iter 2.

Sharding: data-parallel over batch: core c handles n in {2c, 2c+1} (NL=2), all
32 a's.  Groups g = nl*8 + j (j = a-block of 4); partitions = (aL, b).

v2 key changes vs v1:
  * U^T is produced with a PE transpose (matmul vs identity) + 16 small
    stripe copies into a pre-zeroed block-sparse SBUF tile UTS
    [(gL kq), (aL, gL, b)] -- the v1 SBUF->DRAM->SBUF partition-regroup
    round trip (~7us/iter of serialized DMA latency) is gone.
  * L matmuls contract over (gL kq)=128 with a static dense M^T operand
    (MTD); cross-group terms vanish against UTS's static zeros.
  * Both batch-halves are processed in single wide DVE ops (half the
    instruction count on the serial DVE dependency chain).
  * exp(L) is emitted right after each lp tile's matmuls (tail of the
    same iteration) so it runs off the critical path.
  * wga/wws shrunk 4x (q-broadcast moved on-device via stride-0 views).
"""

import numpy as np
import ml_dtypes

import concourse.bass as bass
import concourse.bacc as bacc
import concourse.mybir as mybir
import concourse.tile as tile
from concourse.bass_utils import run_bass_kernel_spmd

F32 = mybir.dt.float32
F16 = mybir.dt.float16
BF16 = mybir.dt.bfloat16

A, B, P, ITERS = 32, 32, 4, 3
PS = P * P                      # 16
BATCH, OH, OW = 16, 16, 16
HW = OH * OW                    # 256
NCORES = 8
NL = BATCH // NCORES            # 2 local batch items per core
J = A // 4                      # 8 j-blocks of 4 a's
G = J * NL                      # 16 groups (g = nl*8 + j)
EPS = 1e-8

AF = mybir.ActivationFunctionType
ALU = mybir.AluOpType
AX = mybir.AxisListType


# ---------------------------------------------------------------- device code
import os as _os
_STOP = _os.environ.get("K_STOP", "")


def _emit(tc, mtd, xh16, xl16, wga, wws, iden, o32):
    nc = tc.nc

    dbg_view = o32.rearrange("n a b k -> (n a b k)") \
                  .rearrange("(p f) -> p f", f=256)

    def dump(src):
        # debug: copy a (128, 256) fp32 AP to the output
        nc.sync.dma_start(out=dbg_view, in_=src)

    with (
        tc.tile_pool(name="inp", bufs=1) as inp,
        tc.tile_pool(name="state", bufs=1) as state,
        tc.tile_pool(name="work", bufs=3) as work,
        tc.tile_pool(name="small", bufs=2) as small,
        tc.tile_pool(name="elp", bufs=4) as elp,
        tc.tile_pool(name="lps", bufs=2, space="PSUM") as lps_pool,
        tc.tile_pool(name="mbps", bufs=1, space="PSUM") as mbps_pool,
    ):
        # ---------------- constants + zeroed UTS first (Pool engine, no deps)
        ones_bf = inp.tile([128, 128], BF16, tag="ones_bf")
        nc.gpsimd.memset(ones_bf[:], 1.0)
        onecol = inp.tile([128, 1], BF16, tag="onecol")
        nc.gpsimd.memset(onecol[:], 1.0)
        epsc = inp.tile([128, 1], F32, tag="epsc")
        nc.gpsimd.memset(epsc[:], EPS)
        # UTS[g4]: [(g4' kq32), (bi, aL, b)] block-sparse U^T with kq padded
        # to 32 so every stripe is 32-partition aligned (engine SBUF access
        # granularity).  One tile per g4 so the four stripe copies can run
        # on different engines without same-tile write serialization.
        # Zeros are static; junk in kq>=16 rows is killed by MTD32's static
        # zero rows.
        UTS = []
        for g4 in range(4):
            row = []
            for hc in range(2):
                u = inp.tile([128, 2 * 4 * 32], F16, tag=f"uts{g4}{hc}")
                nc.gpsimd.memset(u[:], 0.0)
                row.append(u)
            UTS.append(row)
        # per-(iteration, half) Ubar tiles with kq padded to 32; pad columns
        # are zeroed once here and never written again.
        UB = []
        for i in range(2):
            row = []
            for H in range(2):
                u = inp.tile([128, 8 * 32], F16, tag=f"ub{i}{H}")
                nc.gpsimd.memset(u[:], 0.0)
                row.append(u)
            UB.append(row)

        # ---------------- persistent inputs in SBUF
        # x (h/l bf16 split) on the sync queue -- needed first.
        Xh = {}
        Xl = {}
        for ch in range(2):
            th = inp.tile([128, NL * A * PS], BF16, tag=f"xh{ch}")
            nc.sync.dma_start(
                out=th[:].rearrange("p (n c) -> p n c", n=NL),
                in_=xh16[:, ch * 128:(ch + 1) * 128, :].rearrange(
                    "n p c -> p n c"))
            tl = inp.tile([128, NL * A * PS], BF16, tag=f"xl{ch}")
            nc.sync.dma_start(
                out=tl[:].rearrange("p (n c) -> p n c", n=NL),
                in_=xl16[:, ch * 128:(ch + 1) * 128, :].rearrange(
                    "n p c -> p n c"))
            for nl in range(NL):
                Xh[nl, ch] = th[:, nl * A * PS:(nl + 1) * A * PS]
                Xl[nl, ch] = tl[:, nl * A * PS:(nl + 1) * A * PS]

        # weights (vector queue) + M^T/identity (scalar queue): needed later,
        # loaded concurrently with the t=0 Mb matmuls.
        GA = inp.tile([128, G * 16], F32, tag="ga")
        nc.gpsimd.dma_start(out=GA[:], in_=wga[:, :])
        WS = inp.tile([128, G * 16], F32, tag="ws")
        nc.gpsimd.dma_start(out=WS[:], in_=wws[:, :])
        IDEN = inp.tile([128, 128], F16, tag="iden")
        nc.scalar.dma_start(out=IDEN[:], in_=iden[:, :])
        # MTD32[(g4 kq32), ((bi, ch, aL), h)] = M_{g=bi*4+g4}[ch*128+h, aL, kq]
        # for kq<16; rows kq>=16 are zero (host-prepared; plain full-tile DMA
        # -- partition-subset DMA views mis-lower to flat descriptors).
        # Emitted on the sync queue AFTER the x loads so the 1MB transfer
        # cannot jump ahead of them on the shared DMA engines.
        MTD32 = inp.tile([128, 32 * 128], F16, tag="mtd32")
        nc.sync.dma_start(out=MTD32[:], in_=mtd[:, :])

        # Preload the combined exp+ln activation table set once.
        from concourse.hw_specs import get_activation_tables
        _tables = list(get_activation_tables(nc.m.arch).items())
        _set_id = next(i for i, (nm, fns) in enumerate(_tables)
                       if AF.Exp in fns and AF.Ln in fns)
        nc.scalar.add_instruction(mybir.InstLoadActFuncSet(
            name=nc.get_next_instruction_name(),
            ins=[], outs=[], act_func_set_id=_set_id))

        el_tiles = {}

        for t in range(ITERS):
            # ---------------- Mb matmuls (per-half PSUM tiles)
            mb = [mbps_pool.tile([128, 512], F32, tag=f"mb{H}", name=f"mb{H}")
                  for H in range(2)]
            den_ps = None
            if t > 0:
                den_ps = mbps_pool.tile([128, 32], F32, tag="den")
            for g in range(G):
                nl, j = g // J, g % J
                bi, gi = g // 4, g % 4
                H, gL = g // 8, g % 8
                out_g = mb[H][:, gL * 64:(gL + 1) * 64]
                for ch in range(2):
                    if t == 0:
                        lhsT = ones_bf[:]
                    else:
                        lhsT = el_tiles[bi][:, ch * 512 + gi * 128:
                                            ch * 512 + (gi + 1) * 128]
                        nc.tensor.matmul(
                            den_ps[:, g * 2 + ch:g * 2 + ch + 1],
                            lhsT, onecol[:], start=True, stop=True)
                    rx = Xh[nl, ch][:].rearrange(
                        "p (a kq) -> p a kq", kq=PS)[:, 4 * j:4 * j + 4, :]
                    nc.tensor.matmul(out_g, lhsT, rx,
                                     start=(ch == 0), stop=False)
                    rxl = Xl[nl, ch][:].rearrange(
                        "p (a kq) -> p a kq", kq=PS)[:, 4 * j:4 * j + 4, :]
                    nc.tensor.matmul(out_g, lhsT, rxl,
                                     start=False, stop=(ch == 1))

            def _dump32(src_ap, n=256):
                dmp = state.tile([128, 256], F32, tag="dmp")
                nc.gpsimd.memset(dmp[:], 0.0)
                nc.vector.tensor_copy(dmp[:, 0:n], src_ap)
                dump(dmp[:])

            if _STOP == f"t{t}mb":
                _dump32(mb[0][:, 0:256])
                return

            # ---------------- per-half routing chains
            # den folded out of the extraction: mbar holds the UNNORMALIZED
            # Msum (f32); normalization enters via n2 *= recd^2 and
            # fr = f * recd.
            recd = {}
            r2 = {}

            # ---- extraction: pure diagonal copies (h0 -> DVE, h1 -> Act)
            mbar = [state.tile([128, 128], F32, tag=f"mbar{t}{H}", name=f"mbar{t}{H}")
                    for H in range(2)]
            mview = [mbar[H][:].rearrange("p (g kq) -> p g kq", kq=PS)
                     for H in range(2)]
            for H in range(2):
                mbv = mb[H][:].rearrange("p (g c) -> p g c", c=64)
                for aL in range(4):
                    src_ = mbv[aL * 32:(aL + 1) * 32, :,
                               aL * 16:aL * 16 + 16]
                    dst_ = mview[H][aL * 32:(aL + 1) * 32]
                    if H == 0:
                        if t == 0:
                            nc.vector.tensor_scalar_mul(dst_, src_, 1.0 / HW)
                        else:
                            nc.vector.tensor_copy(dst_, src_)
                    else:
                        sc = (1.0 / HW) if t == 0 else 1.0
                        nc.scalar.activation(dst_, src_, AF.Identity,
                                             scale=sc)

            if t > 0:
                for H in range(2):
                    dview = den_ps[:, H * 16:(H + 1) * 16].rearrange(
                        "p (g c) -> p g c", c=2)
                    dcp = small.tile([128, 8], F32, tag=f"dcp{H}")
                    nc.vector.tensor_copy(dcp[:], dview[:, :, 1])
                    dsum = small.tile([128, 8], F32, tag=f"dsum{H}")
                    nc.vector.tensor_add(dsum[:], dview[:, :, 0], dcp[:])
                    rc = small.tile([128, 8], F32, tag=f"recd{H}")
                    nc.vector.reciprocal(rc[:], dsum[:])
                    recd[H] = rc
                    rq = small.tile([128, 8], F32, tag=f"r2{H}")
                    nc.vector.tensor_mul(rq[:], rc[:], rc[:])
                    r2[H] = rq

            if _STOP == f"t{t}mbar":
                _dump32(mbar[0][:, 0:128], n=128)
                return

            eng = {0: nc.vector, 1: nc.gpsimd}

            if t < 2:
                # ---- Z' = G @ Msum  (h0 on DVE, h1 on Pool)
                z = [state.tile([128, 128], F32, tag=f"z{t}{H}", name=f"z{t}{H}")
                     for H in range(2)]
                tz = {}
                for H in range(2):
                    tzt = work.tile([128, 512], F32, tag=f"tz{H}")
                    tz[H] = tzt
                    tzv = tzt[:].rearrange("p (g kp k q) -> p g kp k q",
                                           kp=4, k=4, q=4)
                    gav = GA[:, H * 128:(H + 1) * 128].rearrange(
                        "p (g kp k) -> p g kp k", kp=4, k=4).unsqueeze(4) \
                        .broadcast_to((128, 8, 4, 4, 4))
                    min1 = mview[H].rearrange(
                        "p g (kp q) -> p g kp q", q=4) \
                        .unsqueeze(3).broadcast_to((128, 8, 4, 4, 4))
                    nc.vector.tensor_tensor(tzv, gav, min1, op=ALU.mult)
                t01 = {}
                t23 = {}
                for H in range(2):
                    tzs = tz[H][:].rearrange("p (g kp k q) -> p kp g k q",
                                             kp=4, k=4, q=4)
                    a = work.tile([128, 128], F32, tag=f"t01{H}")
                    nc.vector.tensor_tensor(
                        a[:].rearrange("p (g k q) -> p g k q", k=4, q=4),
                        tzs[:, 0], tzs[:, 1], op=ALU.add)
                    t01[H] = a
                    b = work.tile([128, 128], F32, tag=f"t23{H}")
                    nc.vector.tensor_tensor(
                        b[:].rearrange("p (g k q) -> p g k q", k=4, q=4),
                        tzs[:, 2], tzs[:, 3], op=ALU.add)
                    t23[H] = b
                for H in range(2):
                    nc.vector.tensor_add(z[H][:], t01[H][:], t23[H][:])
                vec = z
            else:
                # ---- final S' = W @ Msum (add tree over k; h0 DVE, h1 Pool)
                s = [state.tile([128, 128], F32, tag=f"sfin{H}", name=f"sfin{H}")
                     for H in range(2)]
                for H in range(2):
                    tst = work.tile([128, 512], F32, tag=f"tsf{H}")
                    tsv = tst[:].rearrange("p (g k pp q) -> p g k pp q",
                                           k=4, pp=4, q=4)
                    wsv = WS[:, H * 128:(H + 1) * 128].rearrange(
                        "p (g k pp) -> p g k pp", k=4, pp=4).unsqueeze(4) \
                        .broadcast_to((128, 8, 4, 4, 4))
                    min2 = mview[H].rearrange(
                        "p g (k q) -> p g k q", q=4) \
                        .unsqueeze(3).broadcast_to((128, 8, 4, 4, 4))
                    nc.vector.tensor_tensor(tsv, wsv, min2, op=ALU.mult)
                    tsk = tst[:].rearrange("p (g k pq) -> p g k pq",
                                           k=4, pq=16)
                    s1 = work.tile([128, 128], F32, tag=f"s1{H}")
                    s1v = s1[:].rearrange("p (g pq) -> p g pq", pq=16)
                    nc.vector.tensor_tensor(s1v, tsk[:, :, 0], tsk[:, :, 1],
                                            op=ALU.add)
                    s2 = work.tile([128, 128], F32, tag=f"s2{H}")
                    s2v = s2[:].rearrange("p (g pq) -> p g pq", pq=16)
                    nc.vector.tensor_tensor(s2v, tsk[:, :, 2], tsk[:, :, 3],
                                            op=ALU.add)
                    nc.vector.tensor_add(s[H][:], s1[:], s2[:])
                vec = s

            # ---- n2 = |.|^2 (per half; reduces on DVE only)
            n2 = {}
            for H in range(2):
                mzt = work.tile([128, 128], F32, tag=f"mz{H}")
                if t < 2:
                    nc.vector.tensor_mul(mzt[:], mbar[H][:], vec[H][:])
                else:
                    nc.vector.tensor_mul(mzt[:], vec[H][:], vec[H][:])
                n2t = small.tile([128, 8], F32, tag=f"n2{H}")
                nc.vector.tensor_reduce(
                    out=n2t[:],
                    in_=mzt[:].rearrange("p (g kq) -> p g kq", kq=PS),
                    op=ALU.add, axis=AX.X)
                if t > 0:
                    n2s = small.tile([128, 8], F32, tag=f"n2s{H}")
                    nc.vector.tensor_mul(n2s[:], n2t[:], r2[H][:])
                    n2t = n2s
                n2[H] = n2t

            # ---- squash factor fr = recd * n2/(1+n2)/sqrt(n2+eps)
            fr = {}
            for H in range(2):
                tln = small.tile([128, 8], F32, tag=f"tln{H}")
                nc.scalar.activation(tln[:], n2[H][:], AF.Ln, bias=epsc[:])
                rr = small.tile([128, 8], F32, tag=f"rr{H}")
                nc.scalar.activation(rr[:], tln[:], AF.Exp, scale=-0.5)
                dd = small.tile([128, 8], F32, tag=f"dd{H}")
                nc.vector.tensor_scalar_add(dd[:], n2[H][:], 1.0)
                rec = small.tile([128, 8], F32, tag=f"rec{H}")
                nc.vector.reciprocal(rec[:], dd[:])
                ff = small.tile([128, 8], F32, tag=f"ff{H}")
                nc.vector.tensor_mul(ff[:], n2[H][:], rec[:])
                ff2 = small.tile([128, 8], F32, tag=f"ff2{H}")
                nc.vector.tensor_mul(ff2[:], ff[:], rr[:])
                if t > 0:
                    frt = small.tile([128, 8], F32, tag=f"fr{H}")
                    nc.vector.tensor_mul(frt[:], ff2[:], recd[H][:])
                    fr[H] = frt
                else:
                    fr[H] = ff2

            if t == 2:
                # ---- output Pout = fr * S'
                for H in range(2):
                    outsb = state.tile([128, 128], F32, tag=f"outsb{H}")
                    frbc = fr[H][:].unsqueeze(2).broadcast_to((128, 8, PS))
                    nc.vector.tensor_tensor(
                        outsb[:].rearrange("p (g kq) -> p g kq", kq=PS),
                        s[H][:].rearrange("p (g kq) -> p g kq", kq=PS),
                        frbc, op=ALU.mult)
                    dst_o = o32[H].rearrange("(jj aL) b kq -> (aL b) jj kq",
                                             jj=J)
                    nc.sync.dma_start(
                        out=dst_o,
                        in_=outsb[:].rearrange("p (g kq) -> p g kq", kq=PS))
                continue

            # ---- U = fr*Z' ; Ubar += U; transpose + stage + stripe
            # copies per half so bi0/1's L matmuls never wait on half 1
            uthp = mbps_pool.tile([128, 512], F16, tag="uthp")
            ust = work.tile([128, 512], F16, tag="ust")
            for H in range(2):
                ubv = UB[t][H][:].rearrange("p (g kq2) -> p g kq2",
                                            kq2=32)[:, :, 0:PS]
                zv = z[H][:].rearrange("p (g kq) -> p g kq", kq=PS)
                frbc = fr[H][:].unsqueeze(2).broadcast_to((128, 8, PS))
                if t == 0:
                    nc.vector.tensor_tensor(ubv, zv, frbc, op=ALU.mult)
                else:
                    u16 = work.tile([128, 128], F16, tag=f"u16{H}")
                    u16v = u16[:].rearrange("p (g kq) -> p g kq", kq=PS)
                    nc.vector.tensor_tensor(u16v, zv, frbc, op=ALU.mult)
                    ub0v = UB[0][H][:].rearrange("p (g kq2) -> p g kq2",
                                                 kq2=32)[:, :, 0:PS]
                    nc.vector.tensor_tensor(ubv, ub0v, u16v, op=ALU.add)
                for half in range(2):
                    bi = H * 2 + half
                    nc.tensor.transpose(
                        uthp[:, bi * 128:(bi + 1) * 128],
                        UB[t][H][:, half * 128:(half + 1) * 128], IDEN[:])
                nc.scalar.activation(ust[:, H * 256:(H + 1) * 256],
                                     uthp[:, H * 256:(H + 1) * 256],
                                     AF.Identity)
                for g4 in range(4):
                    srcu = ust[g4 * 32:(g4 + 1) * 32,
                               H * 256:(H + 1) * 256]
                    dstu = UTS[g4][H][g4 * 32:(g4 + 1) * 32, :]
                    nc.vector.tensor_copy(dstu, srcu)

            if _STOP == f"t{t}ub":
                _dump32(UB[t][0][:, 0:256])
                return
            if _STOP == f"t{t}uts":
                _dump32(UTS[0][0][:, 0:256])
                return
            if _STOP == f"t{t}uthp":
                _dump32(uthp[:, 0:256])
                return

            # ---- L matmuls: L[h, (aL b)] = sum_(g4 kq32) MTD32 * UTS
            lp_tiles = {}
            for bi in range(4):
                lp = lps_pool.tile([128, 1024], F32, tag="lps")
                lp_tiles[bi] = lp
                for gi in range(4):
                    for ch in range(2):
                        for aL in range(4):
                            lhsT = MTD32[:, ((bi * 2 + ch) * 4 + aL) * 128:
                                         ((bi * 2 + ch) * 4 + aL + 1) * 128]
                            rhs = UTS[gi][bi // 2][
                                :, ((bi % 2) * 4 + aL) * 32:
                                ((bi % 2) * 4 + aL + 1) * 32]
                            nc.tensor.matmul(
                                lp[:, ch * 512 + gi * 128 + aL * 32:
                                   ch * 512 + gi * 128 + (aL + 1) * 32],
                                lhsT, rhs, start=True, stop=True)
                # exp of this lp right away (off the critical path)
                el = elp.tile([128, 1024], BF16, tag="el")
                nc.scalar.activation(el[:], lp[:], AF.Exp)
                el_tiles[bi] = el
            if _STOP == f"t{t}lp":
                _dump32(lp_tiles[0][:, 0:256])
                return


def _build_kernel():
    nc = bacc.Bacc("TRN2", target_bir_lowering=False, debug=False,
                   num_devices=NCORES)
    mtd = nc.dram_tensor("mtd", [128, 32 * 128], F16,
                         kind="ExternalInput").ap()
    xh16 = nc.dram_tensor("xh16", [NL, HW, A * PS], BF16,
                          kind="ExternalInput").ap()
    xl16 = nc.dram_tensor("xl16", [NL, HW, A * PS], BF16,
                          kind="ExternalInput").ap()
    wga = nc.dram_tensor("wga", [128, G * 16], F32, kind="ExternalInput").ap()
    wws = nc.dram_tensor("wws", [128, G * 16], F32, kind="ExternalInput").ap()
    iden = nc.dram_tensor("iden", [128, 128], F16, kind="ExternalInput").ap()
    o32 = nc.dram_tensor("o32", [NL, A, B, PS], F32,
                         kind="ExternalOutput").ap()

    with tile.TileContext(nc) as tc:
        _emit(tc, mtd, xh16, xl16, wga, wws, iden, o32)

    nc.compile()
    return nc


# ---------------------------------------------------------------- host side
def _host_weights(weights):
    W = np.asarray(weights, np.float32)                # (A, B, P, P)
    Gm = np.einsum("abpk,abpl->abkl", W, W)            # (A, B, 4, 4)
    Gsw = np.swapaxes(Gm, 2, 3)                        # Gsw[a,b,kp,k]=Gm[k,kp]
    Wsw = np.swapaxes(W, 2, 3)                         # Wsw[a,b,k,pp]=W[pp,k]

    wga = np.zeros((4, B, G, 4, 4), np.float32)        # (aL,b,g,kp,k)
    wws = np.zeros((4, B, G, 4, 4), np.float32)        # (aL,b,g,k,pp)
    for g in range(G):
        j = g % J                                      # g = nl*8 + j
        wga[:, :, g] = Gsw[4 * j:4 * j + 4]
        wws[:, :, g] = Wsw[4 * j:4 * j + 4]
    wga = wga.reshape(4 * B, G * 16)
    wws = wws.reshape(4 * B, G * 16)
    return wga.astype(np.float32), wws.astype(np.float32)


def _host_prep(x, weights):
    xr = np.asarray(x, np.float32).reshape(BATCH, HW, A, PS)
    wga, wws = _host_weights(weights)
    iden = np.eye(128, dtype=np.float16)

    in_maps = []
    for c in range(NCORES):
        xc = xr[c * NL:(c + 1) * NL]                   # (NL, HW, A, PS)
        xh = xc.astype(ml_dtypes.bfloat16)
        xl = (xc - xh.astype(np.float32)).astype(ml_dtypes.bfloat16)
        # mtd[(g4 kq32), ((bi, ch, aL), h)] = xc[nl, ch*128+h, 4j+aL, kq]
        # for kq<16 (zeros at kq>=16), with g = bi*4 + g4 = nl*8 + j.
        xj = xc.reshape(NL, 2, 128, J, 4, PS)          # nl,ch,h,j,aL,kq
        mt = np.zeros((4, 32, 4, 2, 4, 128), np.float32)
        for bi in range(4):
            nl = bi // 2
            for g4 in range(4):
                j = (bi % 2) * 4 + g4
                # xj[nl, :, :, j] is (ch, h, aL, kq) -> (kq, ch, aL, h)
                mt[g4, 0:PS, bi] = xj[nl, :, :, j].transpose(3, 0, 2, 1)
        mtdc = mt.reshape(128, 4096).astype(np.float16)
        in_maps.append({
            "mtd": np.ascontiguousarray(mtdc),
            "xh16": np.ascontiguousarray(xh.reshape(NL, HW, A * PS)),
            "xl16": np.ascontiguousarray(xl.reshape(NL, HW, A * PS)),
            "wga": wga,
            "wws": wws,
            "iden": iden,
        })
    return in_maps


_NC_CACHE = {}


def kernel(x, weights):
    if "nc" not in _NC_CACHE:
        _NC_CACHE["nc"] = _build_kernel()
    nc = _NC_CACHE["nc"]
    in_maps = _host_prep(x, weights)
    res = run_bass_kernel_spmd(nc, in_maps, list(range(NCORES)))
    out = np.concatenate([res.results[c]["o32"] for c in range(NCORES)],
                         axis=0)
    return out.astype(np.float32)


# revision 52
# speedup vs baseline: 1.3789x; 1.0086x over previous
"""DenseCapsLayer Trainium2 kernel (v2 — no DRAM round-trip).

Math (per (n, a) pair; A=32 input capsule types, B=32 output, P=4, hw=256):
  votes v[h,b] = W[a,b] @ M[h]  (4x4 matmuls) -- NEVER materialized (256MB).
  Routing reduces to small per-pair contractions:
    Mbar[b]   = sum_h c[h,b] * M[h]          (c = softmax over h of L)
    S[b]      = W[a,b] @ Mbar[b]
    n2[b]     = |S[b]|^2 = <Mbar[b], G[a,b] @ Mbar[b]>,  G = W^T W  (host-precomputed)
    Pout[b]   = f(n2) * S[b]                  (squash factor f)
    U[b]      = W^T Pout[b] = f * G @ Mbar[b]
    L        += M @ U^T  (so L_t = M @ Ubar_t^T with Ubar = cumulative sum of U)
  Final output = Pout at iter 2.

Sharding: data-parallel over batch: core c handles n in {2c, 2c+1} (NL=2), all
32 a's.  Groups g = nl*8 + j (j = a-block of 4); partitions = (aL, b).

v2 key changes vs v1:
  * U^T is produced with a PE transpose (matmul vs identity) + 16 small
    stripe copies into a pre-zeroed block-sparse SBUF tile UTS
    [(gL kq), (aL, gL, b)] -- the v1 SBUF->DRAM->SBUF partition-regroup
    round trip (~7us/iter of serialized DMA latency) is gone.
  * L matmuls contract over (gL kq)=128 with a static dense M^T operand
    (MTD); cross-group terms vanish against UTS's static zeros.
  * Both batch-halves are processed in single wide DVE ops (half the
    instruction count on the serial DVE dependency chain).
  * exp(L) is emitted right after each lp tile's matmuls (tail of the
    same iteration) so it runs off the critical path.
  * wga/wws shrunk 4x (q-broadcast moved on-device via stride-0 views).
"""

import numpy as np
import ml_dtypes

import concourse.bass as bass
import concourse.bacc as bacc
import concourse.mybir as mybir
import concourse.tile as tile
from concourse.bass_utils import run_bass_kernel_spmd

F32 = mybir.dt.float32
F16 = mybir.dt.float16
BF16 = mybir.dt.bfloat16

A, B, P, ITERS = 32, 32, 4, 3
PS = P * P                      # 16
BATCH, OH, OW = 16, 16, 16
HW = OH * OW                    # 256
NCORES = 8
NL = BATCH // NCORES            # 2 local batch items per core
J = A // 4                      # 8 j-blocks of 4 a's
G = J * NL                      # 16 groups (g = nl*8 + j)
EPS = 1e-8

AF = mybir.ActivationFunctionType
ALU = mybir.AluOpType
AX = mybir.AxisListType


# ---------------------------------------------------------------- device code
import os as _os
_STOP = _os.environ.get("K_STOP", "")


def _emit(tc, mtd, xh16, xl16, wga, wws, iden, o32):
    nc = tc.nc

    dbg_view = o32.rearrange("n a b k -> (n a b k)") \
                  .rearrange("(p f) -> p f", f=256)

    def dump(src):
        # debug: copy a (128, 256) fp32 AP to the output
        nc.sync.dma_start(out=dbg_view, in_=src)

    with (
        tc.tile_pool(name="inp", bufs=1) as inp,
        tc.tile_pool(name="state", bufs=1) as state,
        tc.tile_pool(name="work", bufs=3) as work,
        tc.tile_pool(name="small", bufs=2) as small,
        tc.tile_pool(name="elp", bufs=4) as elp,
        tc.tile_pool(name="lps", bufs=2, space="PSUM") as lps_pool,
        tc.tile_pool(name="mbps", bufs=1, space="PSUM") as mbps_pool,
    ):
        # ---------------- constants + zeroed UTS first (Pool engine, no deps)
        ones_bf = inp.tile([128, 128], BF16, tag="ones_bf")
        nc.gpsimd.memset(ones_bf[:], 1.0)
        onecol = inp.tile([128, 1], BF16, tag="onecol")
        nc.gpsimd.memset(onecol[:], 1.0)
        epsc = inp.tile([128, 1], F32, tag="epsc")
        nc.gpsimd.memset(epsc[:], EPS)
        # UTS[g4]: [(g4' kq32), (bi, aL, b)] block-sparse U^T with kq padded
        # to 32 so every stripe is 32-partition aligned (engine SBUF access
        # granularity).  One tile per g4 so the four stripe copies can run
        # on different engines without same-tile write serialization.
        # Zeros are static; junk in kq>=16 rows is killed by MTD32's static
        # zero rows.
        UTS = []
        for g4 in range(4):
            row = []
            for hc in range(2):
                u = inp.tile([128, 2 * 4 * 32], F16, tag=f"uts{g4}{hc}")
                nc.gpsimd.memset(u[:], 0.0)
                row.append(u)
            UTS.append(row)
        # per-(iteration, half) Ubar tiles with kq padded to 32; pad columns
        # are zeroed once here and never written again.
        UB = []
        for i in range(2):
            row = []
            for H in range(2):
                u = inp.tile([128, 8 * 32], F16, tag=f"ub{i}{H}")
                nc.gpsimd.memset(u[:], 0.0)
                row.append(u)
            UB.append(row)

        # ---------------- persistent inputs in SBUF
        # x (h/l bf16 split) on the sync queue -- needed first.
        Xh = {}
        Xl = {}
        for ch in range(2):
            th = inp.tile([128, NL * A * PS], BF16, tag=f"xh{ch}")
            nc.sync.dma_start(
                out=th[:].rearrange("p (n c) -> p n c", n=NL),
                in_=xh16[:, ch * 128:(ch + 1) * 128, :].rearrange(
                    "n p c -> p n c"))
            tl = inp.tile([128, NL * A * PS], BF16, tag=f"xl{ch}")
            nc.sync.dma_start(
                out=tl[:].rearrange("p (n c) -> p n c", n=NL),
                in_=xl16[:, ch * 128:(ch + 1) * 128, :].rearrange(
                    "n p c -> p n c"))
            for nl in range(NL):
                Xh[nl, ch] = th[:, nl * A * PS:(nl + 1) * A * PS]
                Xl[nl, ch] = tl[:, nl * A * PS:(nl + 1) * A * PS]

        # weights (vector queue) + M^T/identity (scalar queue): needed later,
        # loaded concurrently with the t=0 Mb matmuls.
        GA = inp.tile([128, G * 16], F32, tag="ga")
        nc.gpsimd.dma_start(out=GA[:], in_=wga[:, :])
        WS = inp.tile([128, G * 16], F32, tag="ws")
        nc.gpsimd.dma_start(out=WS[:], in_=wws[:, :])
        IDEN = inp.tile([128, 128], F16, tag="iden")
        nc.scalar.dma_start(out=IDEN[:], in_=iden[:, :])
        # MTD32[(g4 kq32), ((bi, ch, aL), h)] = M_{g=bi*4+g4}[ch*128+h, aL, kq]
        # for kq<16; rows kq>=16 are zero (host-prepared; plain full-tile DMA
        # -- partition-subset DMA views mis-lower to flat descriptors).
        # Emitted on the sync queue AFTER the x loads so the 1MB transfer
        # cannot jump ahead of them on the shared DMA engines.
        MTD32 = inp.tile([128, 32 * 128], F16, tag="mtd32")
        nc.sync.dma_start(out=MTD32[:], in_=mtd[:, :])

        # Preload the combined exp+ln activation table set once.
        from concourse.hw_specs import get_activation_tables
        _tables = list(get_activation_tables(nc.m.arch).items())
        _set_id = next(i for i, (nm, fns) in enumerate(_tables)
                       if AF.Exp in fns and AF.Ln in fns)
        nc.scalar.add_instruction(mybir.InstLoadActFuncSet(
            name=nc.get_next_instruction_name(),
            ins=[], outs=[], act_func_set_id=_set_id))

        el_tiles = {}

        for t in range(ITERS):
            # ---------------- Mb matmuls (per-half PSUM tiles)
            mb = [mbps_pool.tile([128, 512], F32, tag=f"mb{H}", name=f"mb{H}")
                  for H in range(2)]
            den_ps = None
            if t > 0:
                den_ps = mbps_pool.tile([128, 32], F32, tag="den")
            mb_iter = [(g, ch) for g in range(G) for ch in range(2)]
            for g, ch in mb_iter:
                nl, j = g // J, g % J
                bi, gi = g // 4, g % 4
                H, gL = g // 8, g % 8
                out_g = mb[H][:, gL * 64:(gL + 1) * 64]
                if t == 0:
                    lhsT = ones_bf[:]
                else:
                    lhsT = el_tiles[bi][:, ch * 512 + gi * 128:
                                        ch * 512 + (gi + 1) * 128]
                    nc.tensor.matmul(
                        den_ps[:, g * 2 + ch:g * 2 + ch + 1],
                        lhsT, onecol[:], start=True, stop=True)
                rx = Xh[nl, ch][:].rearrange(
                    "p (a kq) -> p a kq", kq=PS)[:, 4 * j:4 * j + 4, :]
                nc.tensor.matmul(out_g, lhsT, rx,
                                 start=(ch == 0), stop=False,
                                 skip_group_check=True)
                rxl = Xl[nl, ch][:].rearrange(
                    "p (a kq) -> p a kq", kq=PS)[:, 4 * j:4 * j + 4, :]
                nc.tensor.matmul(out_g, lhsT, rxl,
                                 start=False, stop=(ch == 1),
                                 skip_group_check=True)

            def _dump32(src_ap, n=256):
                dmp = state.tile([128, 256], F32, tag="dmp")
                nc.gpsimd.memset(dmp[:], 0.0)
                nc.vector.tensor_copy(dmp[:, 0:n], src_ap)
                dump(dmp[:])

            if _STOP == f"t{t}mb":
                _dump32(mb[0][:, 0:256])
                return

            # ---------------- per-half routing chains
            # den folded out of the extraction: mbar holds the UNNORMALIZED
            # Msum (f32); normalization enters via n2 *= recd^2 and
            # fr = f * recd.
            recd = {}
            r2 = {}

            # ---- extraction: pure diagonal copies (h0 -> DVE, h1 -> Act)
            mbar = [state.tile([128, 128], F32, tag=f"mbar{t}{H}", name=f"mbar{t}{H}")
                    for H in range(2)]
            mview = [mbar[H][:].rearrange("p (g kq) -> p g kq", kq=PS)
                     for H in range(2)]
            for H in range(2):
                mbv = mb[H][:].rearrange("p (g c) -> p g c", c=64)
                for aL in range(4):
                    src_ = mbv[aL * 32:(aL + 1) * 32, :,
                               aL * 16:aL * 16 + 16]
                    dst_ = mview[H][aL * 32:(aL + 1) * 32]
                    if H == 0:
                        if t == 0:
                            nc.vector.tensor_scalar_mul(dst_, src_, 1.0 / HW)
                        else:
                            nc.vector.tensor_copy(dst_, src_)
                    else:
                        sc = (1.0 / HW) if t == 0 else 1.0
                        nc.scalar.activation(dst_, src_, AF.Identity,
                                             scale=sc)

            if t > 0:
                for H in range(2):
                    dview = den_ps[:, H * 16:(H + 1) * 16].rearrange(
                        "p (g c) -> p g c", c=2)
                    dcp = small.tile([128, 8], F32, tag=f"dcp{H}")
                    nc.vector.tensor_copy(dcp[:], dview[:, :, 1])
                    dsum = small.tile([128, 8], F32, tag=f"dsum{H}")
                    nc.vector.tensor_add(dsum[:], dview[:, :, 0], dcp[:])
                    rc = small.tile([128, 8], F32, tag=f"recd{H}")
                    nc.vector.reciprocal(rc[:], dsum[:])
                    recd[H] = rc
                    rq = small.tile([128, 8], F32, tag=f"r2{H}")
                    nc.vector.tensor_mul(rq[:], rc[:], rc[:])
                    r2[H] = rq

            if _STOP == f"t{t}mbar":
                _dump32(mbar[0][:, 0:128], n=128)
                return

            eng = {0: nc.vector, 1: nc.gpsimd}

            if t < 2:
                # ---- Z' = G @ Msum  (h0 on DVE, h1 on Pool)
                z = [state.tile([128, 128], F32, tag=f"z{t}{H}", name=f"z{t}{H}")
                     for H in range(2)]
                tz = {}
                for H in range(2):
                    tzt = work.tile([128, 512], F32, tag=f"tz{H}")
                    tz[H] = tzt
                    tzv = tzt[:].rearrange("p (g kp k q) -> p g kp k q",
                                           kp=4, k=4, q=4)
                    gav = GA[:, H * 128:(H + 1) * 128].rearrange(
                        "p (g kp k) -> p g kp k", kp=4, k=4).unsqueeze(4) \
                        .broadcast_to((128, 8, 4, 4, 4))
                    min1 = mview[H].rearrange(
                        "p g (kp q) -> p g kp q", q=4) \
                        .unsqueeze(3).broadcast_to((128, 8, 4, 4, 4))
                    te = nc.vector if H == 0 else nc.gpsimd
                    te.tensor_tensor(tzv, gav, min1, op=ALU.mult)
                t01 = {}
                t23 = {}
                for H in range(2):
                    tzs = tz[H][:].rearrange("p (g kp k q) -> p kp g k q",
                                             kp=4, k=4, q=4)
                    a = work.tile([128, 128], F32, tag=f"t01{H}")
                    nc.vector.tensor_tensor(
                        a[:].rearrange("p (g k q) -> p g k q", k=4, q=4),
                        tzs[:, 0], tzs[:, 1], op=ALU.add)
                    t01[H] = a
                    b = work.tile([128, 128], F32, tag=f"t23{H}")
                    nc.vector.tensor_tensor(
                        b[:].rearrange("p (g k q) -> p g k q", k=4, q=4),
                        tzs[:, 2], tzs[:, 3], op=ALU.add)
                    t23[H] = b
                for H in range(2):
                    nc.vector.tensor_add(z[H][:], t01[H][:], t23[H][:])
                vec = z
            else:
                # ---- final S' = W @ Msum (add tree over k; h0 DVE, h1 Pool)
                s = [state.tile([128, 128], F32, tag=f"sfin{H}", name=f"sfin{H}")
                     for H in range(2)]
                for H in range(2):
                    tst = work.tile([128, 512], F32, tag=f"tsf{H}")
                    tsv = tst[:].rearrange("p (g k pp q) -> p g k pp q",
                                           k=4, pp=4, q=4)
                    wsv = WS[:, H * 128:(H + 1) * 128].rearrange(
                        "p (g k pp) -> p g k pp", k=4, pp=4).unsqueeze(4) \
                        .broadcast_to((128, 8, 4, 4, 4))
                    min2 = mview[H].rearrange(
                        "p g (k q) -> p g k q", q=4) \
                        .unsqueeze(3).broadcast_to((128, 8, 4, 4, 4))
                    nc.vector.tensor_tensor(tsv, wsv, min2, op=ALU.mult)
                    tsk = tst[:].rearrange("p (g k pq) -> p g k pq",
                                           k=4, pq=16)
                    s1 = work.tile([128, 128], F32, tag=f"s1{H}")
                    s1v = s1[:].rearrange("p (g pq) -> p g pq", pq=16)
                    nc.vector.tensor_tensor(s1v, tsk[:, :, 0], tsk[:, :, 1],
                                            op=ALU.add)
                    s2 = work.tile([128, 128], F32, tag=f"s2{H}")
                    s2v = s2[:].rearrange("p (g pq) -> p g pq", pq=16)
                    nc.vector.tensor_tensor(s2v, tsk[:, :, 2], tsk[:, :, 3],
                                            op=ALU.add)
                    nc.vector.tensor_add(s[H][:], s1[:], s2[:])
                vec = s

            # ---- n2 = |.|^2 (per half; reduces on DVE only)
            n2 = {}
            for H in range(2):
                mzt = work.tile([128, 128], F32, tag=f"mz{H}")
                if t < 2:
                    nc.vector.tensor_mul(mzt[:], mbar[H][:], vec[H][:])
                else:
                    nc.vector.tensor_mul(mzt[:], vec[H][:], vec[H][:])
                n2t = small.tile([128, 8], F32, tag=f"n2{H}")
                nc.vector.tensor_reduce(
                    out=n2t[:],
                    in_=mzt[:].rearrange("p (g kq) -> p g kq", kq=PS),
                    op=ALU.add, axis=AX.X)
                if t > 0:
                    n2s = small.tile([128, 8], F32, tag=f"n2s{H}")
                    nc.vector.tensor_mul(n2s[:], n2t[:], r2[H][:])
                    n2t = n2s
                n2[H] = n2t

            # ---- squash factor fr = recd * n2/(1+n2)/sqrt(n2+eps)
            fr = {}
            for H in range(2):
                tln = small.tile([128, 8], F32, tag=f"tln{H}")
                nc.scalar.activation(tln[:], n2[H][:], AF.Ln, bias=epsc[:])
                rr = small.tile([128, 8], F32, tag=f"rr{H}")
                nc.scalar.activation(rr[:], tln[:], AF.Exp, scale=-0.5)
                dd = small.tile([128, 8], F32, tag=f"dd{H}")
                nc.vector.tensor_scalar_add(dd[:], n2[H][:], 1.0)
                rec = small.tile([128, 8], F32, tag=f"rec{H}")
                nc.vector.reciprocal(rec[:], dd[:])
                ff = small.tile([128, 8], F32, tag=f"ff{H}")
                nc.vector.tensor_mul(ff[:], n2[H][:], rec[:])
                ff2 = small.tile([128, 8], F32, tag=f"ff2{H}")
                nc.vector.tensor_mul(ff2[:], ff[:], rr[:])
                if t > 0:
                    frt = small.tile([128, 8], F32, tag=f"fr{H}")
                    nc.vector.tensor_mul(frt[:], ff2[:], recd[H][:])
                    fr[H] = frt
                else:
                    fr[H] = ff2

            if t == 2:
                # ---- output Pout = fr * S'
                for H in range(2):
                    outsb = state.tile([128, 128], F32, tag=f"outsb{H}")
                    frbc = fr[H][:].unsqueeze(2).broadcast_to((128, 8, PS))
                    nc.vector.tensor_tensor(
                        outsb[:].rearrange("p (g kq) -> p g kq", kq=PS),
                        s[H][:].rearrange("p (g kq) -> p g kq", kq=PS),
                        frbc, op=ALU.mult)
                    dst_o = o32[H].rearrange("(jj aL) b kq -> (aL b) jj kq",
                                             jj=J)
                    nc.sync.dma_start(
                        out=dst_o,
                        in_=outsb[:].rearrange("p (g kq) -> p g kq", kq=PS))
                continue

            # ---- U = fr*Z' ; Ubar += U; transpose + stage + stripe
            # copies per half so bi0/1's L matmuls never wait on half 1
            uthp = mbps_pool.tile([128, 512], F16, tag="uthp")
            ust = work.tile([128, 512], F16, tag="ust")
            for H in range(2):
                ubv = UB[t][H][:].rearrange("p (g kq2) -> p g kq2",
                                            kq2=32)[:, :, 0:PS]
                zv = z[H][:].rearrange("p (g kq) -> p g kq", kq=PS)
                frbc = fr[H][:].unsqueeze(2).broadcast_to((128, 8, PS))
                if t == 0:
                    nc.vector.tensor_tensor(ubv, zv, frbc, op=ALU.mult)
                else:
                    u16 = work.tile([128, 128], F16, tag=f"u16{H}")
                    u16v = u16[:].rearrange("p (g kq) -> p g kq", kq=PS)
                    nc.vector.tensor_tensor(u16v, zv, frbc, op=ALU.mult)
                    ub0v = UB[0][H][:].rearrange("p (g kq2) -> p g kq2",
                                                 kq2=32)[:, :, 0:PS]
                    nc.vector.tensor_tensor(ubv, ub0v, u16v, op=ALU.add)
                for half in range(2):
                    bi = H * 2 + half
                    nc.tensor.transpose(
                        uthp[:, bi * 128:(bi + 1) * 128],
                        UB[t][H][:, half * 128:(half + 1) * 128], IDEN[:])
                nc.scalar.activation(ust[:, H * 256:(H + 1) * 256],
                                     uthp[:, H * 256:(H + 1) * 256],
                                     AF.Identity)
                for g4 in range(4):
                    srcu = ust[g4 * 32:(g4 + 1) * 32,
                               H * 256:(H + 1) * 256]
                    dstu = UTS[g4][H][g4 * 32:(g4 + 1) * 32, :]
                    nc.vector.tensor_copy(dstu, srcu)

            if _STOP == f"t{t}ub":
                _dump32(UB[t][0][:, 0:256])
                return
            if _STOP == f"t{t}uts":
                _dump32(UTS[0][0][:, 0:256])
                return
            if _STOP == f"t{t}uthp":
                _dump32(uthp[:, 0:256])
                return

            # ---- L matmuls: L[h, (aL b)] = sum_(g4 kq32) MTD32 * UTS
            lp_tiles = {}
            for bi in range(4):
                lp = lps_pool.tile([128, 1024], F32, tag="lps")
                lp_tiles[bi] = lp
                for gi in range(4):
                    for ch in range(2):
                        for aL in range(4):
                            lhsT = MTD32[:, ((bi * 2 + ch) * 4 + aL) * 128:
                                         ((bi * 2 + ch) * 4 + aL + 1) * 128]
                            rhs = UTS[gi][bi // 2][
                                :, ((bi % 2) * 4 + aL) * 32:
                                ((bi % 2) * 4 + aL + 1) * 32]
                            nc.tensor.matmul(
                                lp[:, ch * 512 + gi * 128 + aL * 32:
                                   ch * 512 + gi * 128 + (aL + 1) * 32],
                                lhsT, rhs, start=True, stop=True)
                # exp of this lp right away (off the critical path)
                el = elp.tile([128, 1024], BF16, tag="el")
                nc.scalar.activation(el[:], lp[:], AF.Exp)
                el_tiles[bi] = el
            if _STOP == f"t{t}lp":
                _dump32(lp_tiles[0][:, 0:256])
                return


def _build_kernel():
    nc = bacc.Bacc("TRN2", target_bir_lowering=False, debug=False,
                   num_devices=NCORES)
    mtd = nc.dram_tensor("mtd", [128, 32 * 128], F16,
                         kind="ExternalInput").ap()
    xh16 = nc.dram_tensor("xh16", [NL, HW, A * PS], BF16,
                          kind="ExternalInput").ap()
    xl16 = nc.dram_tensor("xl16", [NL, HW, A * PS], BF16,
                          kind="ExternalInput").ap()
    wga = nc.dram_tensor("wga", [128, G * 16], F32, kind="ExternalInput").ap()
    wws = nc.dram_tensor("wws", [128, G * 16], F32, kind="ExternalInput").ap()
    iden = nc.dram_tensor("iden", [128, 128], F16, kind="ExternalInput").ap()
    o32 = nc.dram_tensor("o32", [NL, A, B, PS], F32,
                         kind="ExternalOutput").ap()

    with tile.TileContext(nc) as tc:
        _emit(tc, mtd, xh16, xl16, wga, wws, iden, o32)

    nc.compile()
    return nc


# ---------------------------------------------------------------- host side
def _host_weights(weights):
    W = np.asarray(weights, np.float32)                # (A, B, P, P)
    Gm = np.einsum("abpk,abpl->abkl", W, W)            # (A, B, 4, 4)
    Gsw = np.swapaxes(Gm, 2, 3)                        # Gsw[a,b,kp,k]=Gm[k,kp]
    Wsw = np.swapaxes(W, 2, 3)                         # Wsw[a,b,k,pp]=W[pp,k]

    wga = np.zeros((4, B, G, 4, 4), np.float32)        # (aL,b,g,kp,k)
    wws = np.zeros((4, B, G, 4, 4), np.float32)        # (aL,b,g,k,pp)
    for g in range(G):
        j = g % J                                      # g = nl*8 + j
        wga[:, :, g] = Gsw[4 * j:4 * j + 4]
        wws[:, :, g] = Wsw[4 * j:4 * j + 4]
    wga = wga.reshape(4 * B, G * 16)
    wws = wws.reshape(4 * B, G * 16)
    return wga.astype(np.float32), wws.astype(np.float32)


def _host_prep(x, weights):
    xr = np.asarray(x, np.float32).reshape(BATCH, HW, A, PS)
    wga, wws = _host_weights(weights)
    iden = np.eye(128, dtype=np.float16)

    in_maps = []
    for c in range(NCORES):
        xc = xr[c * NL:(c + 1) * NL]                   # (NL, HW, A, PS)
        xh = xc.astype(ml_dtypes.bfloat16)
        xl = (xc - xh.astype(np.float32)).astype(ml_dtypes.bfloat16)
        # mtd[(g4 kq32), ((bi, ch, aL), h)] = xc[nl, ch*128+h, 4j+aL, kq]
        # for kq<16 (zeros at kq>=16), with g = bi*4 + g4 = nl*8 + j.
        xj = xc.reshape(NL, 2, 128, J, 4, PS)          # nl,ch,h,j,aL,kq
        mt = np.zeros((4, 32, 4, 2, 4, 128), np.float32)
        for bi in range(4):
            nl = bi // 2
            for g4 in range(4):
                j = (bi % 2) * 4 + g4
                # xj[nl, :, :, j] is (ch, h, aL, kq) -> (kq, ch, aL, h)
                mt[g4, 0:PS, bi] = xj[nl, :, :, j].transpose(3, 0, 2, 1)
        mtdc = mt.reshape(128, 4096).astype(np.float16)
        in_maps.append({
            "mtd": np.ascontiguousarray(mtdc),
            "xh16": np.ascontiguousarray(xh.reshape(NL, HW, A * PS)),
            "xl16": np.ascontiguousarray(xl.reshape(NL, HW, A * PS)),
            "wga": wga,
            "wws": wws,
            "iden": iden,
        })
    return in_maps


_NC_CACHE = {}


def kernel(x, weights):
    if "nc" not in _NC_CACHE:
        _NC_CACHE["nc"] = _build_kernel()
    nc = _NC_CACHE["nc"]
    in_maps = _host_prep(x, weights)
    res = run_bass_kernel_spmd(nc, in_maps, list(range(NCORES)))
    out = np.concatenate([res.results[c]["o32"] for c in range(NCORES)],
                         axis=0)
    return out.astype(np.float32)


# revision 55
# speedup vs baseline: 1.3928x; 1.0101x over previous
"""DenseCapsLayer Trainium2 kernel (v2 — no DRAM round-trip).

Math (per (n, a) pair; A=32 input capsule types, B=32 output, P=4, hw=256):
  votes v[h,b] = W[a,b] @ M[h]  (4x4 matmuls) -- NEVER materialized (256MB).
  Routing reduces to small per-pair contractions:
    Mbar[b]   = sum_h c[h,b] * M[h]          (c = softmax over h of L)
    S[b]      = W[a,b] @ Mbar[b]
    n2[b]     = |S[b]|^2 = <Mbar[b], G[a,b] @ Mbar[b]>,  G = W^T W  (host-precomputed)
    Pout[b]   = f(n2) * S[b]                  (squash factor f)
    U[b]      = W^T Pout[b] = f * G @ Mbar[b]
    L        += M @ U^T  (so L_t = M @ Ubar_t^T with Ubar = cumulative sum of U)
  Final output = Pout at iter 2.

Sharding: data-parallel over batch: core c handles n in {2c, 2c+1} (NL=2), all
32 a's.  Groups g = nl*8 + j (j = a-block of 4); partitions = (aL, b).

v2 key changes vs v1:
  * U^T is produced with a PE transpose (matmul vs identity) + 16 small
    stripe copies into a pre-zeroed block-sparse SBUF tile UTS
    [(gL kq), (aL, gL, b)] -- the v1 SBUF->DRAM->SBUF partition-regroup
    round trip (~7us/iter of serialized DMA latency) is gone.
  * L matmuls contract over (gL kq)=128 with a static dense M^T operand
    (MTD); cross-group terms vanish against UTS's static zeros.
  * Both batch-halves are processed in single wide DVE ops (half the
    instruction count on the serial DVE dependency chain).
  * exp(L) is emitted right after each lp tile's matmuls (tail of the
    same iteration) so it runs off the critical path.
  * wga/wws shrunk 4x (q-broadcast moved on-device via stride-0 views).
"""

import numpy as np
import ml_dtypes

import concourse.bass as bass
import concourse.bacc as bacc
import concourse.mybir as mybir
import concourse.tile as tile
from concourse.bass_utils import run_bass_kernel_spmd

F32 = mybir.dt.float32
F16 = mybir.dt.float16
BF16 = mybir.dt.bfloat16

A, B, P, ITERS = 32, 32, 4, 3
PS = P * P                      # 16
BATCH, OH, OW = 16, 16, 16
HW = OH * OW                    # 256
NCORES = 8
NL = BATCH // NCORES            # 2 local batch items per core
J = A // 4                      # 8 j-blocks of 4 a's
G = J * NL                      # 16 groups (g = nl*8 + j)
EPS = 1e-8

AF = mybir.ActivationFunctionType
ALU = mybir.AluOpType
AX = mybir.AxisListType


# ---------------------------------------------------------------- device code
import os as _os
_STOP = _os.environ.get("K_STOP", "")


def _emit(tc, mtd, xh16, xl16, wga, wws, iden, o32):
    nc = tc.nc

    dbg_view = o32.rearrange("n a b k -> (n a b k)") \
                  .rearrange("(p f) -> p f", f=256)

    def dump(src):
        # debug: copy a (128, 256) fp32 AP to the output
        nc.sync.dma_start(out=dbg_view, in_=src)

    with (
        tc.tile_pool(name="inp", bufs=1) as inp,
        tc.tile_pool(name="state", bufs=1) as state,
        tc.tile_pool(name="work", bufs=3) as work,
        tc.tile_pool(name="small", bufs=2) as small,
        tc.tile_pool(name="elp", bufs=4) as elp,
        tc.tile_pool(name="lps", bufs=2, space="PSUM") as lps_pool,
        tc.tile_pool(name="mbps", bufs=1, space="PSUM") as mbps_pool,
    ):
        # ---------------- constants + zeroed UTS first (Pool engine, no deps)
        ones_bf = inp.tile([128, 128], BF16, tag="ones_bf")
        nc.gpsimd.memset(ones_bf[:], 1.0)
        onecol = inp.tile([128, 1], BF16, tag="onecol")
        nc.gpsimd.memset(onecol[:], 1.0)
        epsc = inp.tile([128, 1], F32, tag="epsc")
        nc.gpsimd.memset(epsc[:], EPS)
        # UTS[g4]: [(g4' kq32), (bi, aL, b)] block-sparse U^T with kq padded
        # to 32 so every stripe is 32-partition aligned (engine SBUF access
        # granularity).  One tile per g4 so the four stripe copies can run
        # on different engines without same-tile write serialization.
        # Zeros are static; junk in kq>=16 rows is killed by MTD32's static
        # zero rows.
        UTS = []
        for g4 in range(4):
            row = []
            for hc in range(2):
                u = inp.tile([128, 2 * 4 * 32], F16, tag=f"uts{g4}{hc}")
                nc.gpsimd.memset(u[:], 0.0)
                row.append(u)
            UTS.append(row)
        # per-(iteration, half) Ubar tiles with kq padded to 32; pad columns
        # are zeroed once here and never written again.
        UB = []
        for i in range(2):
            row = []
            for H in range(2):
                u = inp.tile([128, 8 * 32], F16, tag=f"ub{i}{H}")
                nc.gpsimd.memset(u[:], 0.0)
                row.append(u)
            UB.append(row)

        # ---------------- persistent inputs in SBUF
        # x (h/l bf16 split) on the sync queue -- needed first.
        Xh = {}
        Xl = {}
        for ch in range(2):
            th = inp.tile([128, NL * A * PS], BF16, tag=f"xh{ch}")
            nc.sync.dma_start(
                out=th[:].rearrange("p (n c) -> p n c", n=NL),
                in_=xh16[:, ch * 128:(ch + 1) * 128, :].rearrange(
                    "n p c -> p n c"))
            tl = inp.tile([128, NL * A * PS], BF16, tag=f"xl{ch}")
            nc.sync.dma_start(
                out=tl[:].rearrange("p (n c) -> p n c", n=NL),
                in_=xl16[:, ch * 128:(ch + 1) * 128, :].rearrange(
                    "n p c -> p n c"))
            for nl in range(NL):
                Xh[nl, ch] = th[:, nl * A * PS:(nl + 1) * A * PS]
                Xl[nl, ch] = tl[:, nl * A * PS:(nl + 1) * A * PS]

        # weights (vector queue) + M^T/identity (scalar queue): needed later,
        # loaded concurrently with the t=0 Mb matmuls.
        GA = inp.tile([128, G * 16], F32, tag="ga")
        nc.gpsimd.dma_start(out=GA[:], in_=wga[:, :])
        WS = inp.tile([128, G * 16], F32, tag="ws")
        nc.gpsimd.dma_start(out=WS[:], in_=wws[:, :])
        IDEN = inp.tile([128, 128], F16, tag="iden")
        nc.scalar.dma_start(out=IDEN[:], in_=iden[:, :])
        # MTD32[(g4 kq32), ((bi, ch, aL), h)] = M_{g=bi*4+g4}[ch*128+h, aL, kq]
        # for kq<16; rows kq>=16 are zero (host-prepared; plain full-tile DMA
        # -- partition-subset DMA views mis-lower to flat descriptors).
        # Emitted on the sync queue AFTER the x loads so the 1MB transfer
        # cannot jump ahead of them on the shared DMA engines.
        MTD32 = inp.tile([128, 32 * 128], F16, tag="mtd32")
        nc.sync.dma_start(out=MTD32[:], in_=mtd[:, :])

        # Preload the combined exp+ln activation table set once.
        from concourse.hw_specs import get_activation_tables
        _tables = list(get_activation_tables(nc.m.arch).items())
        _set_id = next(i for i, (nm, fns) in enumerate(_tables)
                       if AF.Exp in fns and AF.Ln in fns)
        nc.scalar.add_instruction(mybir.InstLoadActFuncSet(
            name=nc.get_next_instruction_name(),
            ins=[], outs=[], act_func_set_id=_set_id))

        el_tiles = {}

        for t in range(ITERS):
            # ---------------- Mb matmuls (per-half PSUM tiles)
            mb = [mbps_pool.tile([128, 512], F32, tag=f"mb{H}", name=f"mb{H}")
                  for H in range(2)]
            den_ps = None
            if t > 0:
                den_ps = mbps_pool.tile([128, 32], F32, tag="den")
            mb_iter = [(g, ch) for g in range(G) for ch in range(2)]
            for g, ch in mb_iter:
                nl, j = g // J, g % J
                bi, gi = g // 4, g % 4
                H, gL = g // 8, g % 8
                out_g = mb[H][:, gL * 64:(gL + 1) * 64]
                if t == 0:
                    lhsT = ones_bf[:]
                else:
                    lhsT = el_tiles[bi][:, ch * 512 + gi * 128:
                                        ch * 512 + (gi + 1) * 128]
                    nc.tensor.matmul(
                        den_ps[:, g * 2 + ch:g * 2 + ch + 1],
                        lhsT, onecol[:], start=True, stop=True)
                rx = Xh[nl, ch][:].rearrange(
                    "p (a kq) -> p a kq", kq=PS)[:, 4 * j:4 * j + 4, :]
                if t == 0:
                    # full h+l precision for the uniform-c mean
                    nc.tensor.matmul(out_g, lhsT, rx,
                                     start=(ch == 0), stop=False,
                                     skip_group_check=True)
                    rxl = Xl[nl, ch][:].rearrange(
                        "p (a kq) -> p a kq", kq=PS)[:, 4 * j:4 * j + 4, :]
                    nc.tensor.matmul(out_g, lhsT, rxl,
                                     start=False, stop=(ch == 1),
                                     skip_group_check=True)
                else:
                    nc.tensor.matmul(out_g, lhsT, rx,
                                     start=(ch == 0), stop=(ch == 1),
                                     skip_group_check=True)

            def _dump32(src_ap, n=256):
                dmp = state.tile([128, 256], F32, tag="dmp")
                nc.gpsimd.memset(dmp[:], 0.0)
                nc.vector.tensor_copy(dmp[:, 0:n], src_ap)
                dump(dmp[:])

            if _STOP == f"t{t}mb":
                _dump32(mb[0][:, 0:256])
                return

            # ---------------- per-half routing chains
            # den folded out of the extraction: mbar holds the UNNORMALIZED
            # Msum (f32); normalization enters via n2 *= recd^2 and
            # fr = f * recd.
            recd = {}
            r2 = {}

            # ---- extraction: pure diagonal copies (h0 -> DVE, h1 -> Act)
            mbar = [state.tile([128, 128], F32, tag=f"mbar{t}{H}", name=f"mbar{t}{H}")
                    for H in range(2)]
            mview = [mbar[H][:].rearrange("p (g kq) -> p g kq", kq=PS)
                     for H in range(2)]
            for H in range(2):
                mbv = mb[H][:].rearrange("p (g c) -> p g c", c=64)
                for aL in range(4):
                    src_ = mbv[aL * 32:(aL + 1) * 32, :,
                               aL * 16:aL * 16 + 16]
                    dst_ = mview[H][aL * 32:(aL + 1) * 32]
                    if H == 0:
                        if t == 0:
                            nc.vector.tensor_scalar_mul(dst_, src_, 1.0 / HW)
                        else:
                            nc.vector.tensor_copy(dst_, src_)
                    else:
                        sc = (1.0 / HW) if t == 0 else 1.0
                        nc.scalar.activation(dst_, src_, AF.Identity,
                                             scale=sc)

            if t > 0:
                for H in range(2):
                    dview = den_ps[:, H * 16:(H + 1) * 16].rearrange(
                        "p (g c) -> p g c", c=2)
                    dcp = small.tile([128, 8], F32, tag=f"dcp{H}")
                    nc.vector.tensor_copy(dcp[:], dview[:, :, 1])
                    dsum = small.tile([128, 8], F32, tag=f"dsum{H}")
                    nc.vector.tensor_add(dsum[:], dview[:, :, 0], dcp[:])
                    rc = small.tile([128, 8], F32, tag=f"recd{H}")
                    nc.vector.reciprocal(rc[:], dsum[:])
                    recd[H] = rc
                    rq = small.tile([128, 8], F32, tag=f"r2{H}")
                    nc.vector.tensor_mul(rq[:], rc[:], rc[:])
                    r2[H] = rq

            if _STOP == f"t{t}mbar":
                _dump32(mbar[0][:, 0:128], n=128)
                return

            eng = {0: nc.vector, 1: nc.gpsimd}

            if t < 2:
                # ---- Z' = G @ Msum  (h0 on DVE, h1 on Pool)
                z = [state.tile([128, 128], F32, tag=f"z{t}{H}", name=f"z{t}{H}")
                     for H in range(2)]
                tz = {}
                for H in range(2):
                    tzt = work.tile([128, 512], F32, tag=f"tz{H}")
                    tz[H] = tzt
                    tzv = tzt[:].rearrange("p (g kp k q) -> p g kp k q",
                                           kp=4, k=4, q=4)
                    gav = GA[:, H * 128:(H + 1) * 128].rearrange(
                        "p (g kp k) -> p g kp k", kp=4, k=4).unsqueeze(4) \
                        .broadcast_to((128, 8, 4, 4, 4))
                    min1 = mview[H].rearrange(
                        "p g (kp q) -> p g kp q", q=4) \
                        .unsqueeze(3).broadcast_to((128, 8, 4, 4, 4))
                    te = nc.vector if H == 0 else nc.gpsimd
                    te.tensor_tensor(tzv, gav, min1, op=ALU.mult)
                t01 = {}
                t23 = {}
                for H in range(2):
                    tzs = tz[H][:].rearrange("p (g kp k q) -> p kp g k q",
                                             kp=4, k=4, q=4)
                    a = work.tile([128, 128], F32, tag=f"t01{H}")
                    nc.vector.tensor_tensor(
                        a[:].rearrange("p (g k q) -> p g k q", k=4, q=4),
                        tzs[:, 0], tzs[:, 1], op=ALU.add)
                    t01[H] = a
                    b = work.tile([128, 128], F32, tag=f"t23{H}")
                    nc.vector.tensor_tensor(
                        b[:].rearrange("p (g k q) -> p g k q", k=4, q=4),
                        tzs[:, 2], tzs[:, 3], op=ALU.add)
                    t23[H] = b
                for H in range(2):
                    nc.vector.tensor_add(z[H][:], t01[H][:], t23[H][:])
                vec = z
            else:
                # ---- final S' = W @ Msum (add tree over k; h0 DVE, h1 Pool)
                s = [state.tile([128, 128], F32, tag=f"sfin{H}", name=f"sfin{H}")
                     for H in range(2)]
                for H in range(2):
                    tst = work.tile([128, 512], F32, tag=f"tsf{H}")
                    tsv = tst[:].rearrange("p (g k pp q) -> p g k pp q",
                                           k=4, pp=4, q=4)
                    wsv = WS[:, H * 128:(H + 1) * 128].rearrange(
                        "p (g k pp) -> p g k pp", k=4, pp=4).unsqueeze(4) \
                        .broadcast_to((128, 8, 4, 4, 4))
                    min2 = mview[H].rearrange(
                        "p g (k q) -> p g k q", q=4) \
                        .unsqueeze(3).broadcast_to((128, 8, 4, 4, 4))
                    nc.vector.tensor_tensor(tsv, wsv, min2, op=ALU.mult)
                    tsk = tst[:].rearrange("p (g k pq) -> p g k pq",
                                           k=4, pq=16)
                    s1 = work.tile([128, 128], F32, tag=f"s1{H}")
                    s1v = s1[:].rearrange("p (g pq) -> p g pq", pq=16)
                    nc.vector.tensor_tensor(s1v, tsk[:, :, 0], tsk[:, :, 1],
                                            op=ALU.add)
                    s2 = work.tile([128, 128], F32, tag=f"s2{H}")
                    s2v = s2[:].rearrange("p (g pq) -> p g pq", pq=16)
                    nc.vector.tensor_tensor(s2v, tsk[:, :, 2], tsk[:, :, 3],
                                            op=ALU.add)
                    nc.vector.tensor_add(s[H][:], s1[:], s2[:])
                vec = s

            # ---- n2 = |.|^2 (per half; reduces on DVE only)
            n2 = {}
            for H in range(2):
                mzt = work.tile([128, 128], F32, tag=f"mz{H}")
                if t < 2:
                    nc.vector.tensor_mul(mzt[:], mbar[H][:], vec[H][:])
                else:
                    nc.vector.tensor_mul(mzt[:], vec[H][:], vec[H][:])
                n2t = small.tile([128, 8], F32, tag=f"n2{H}")
                nc.vector.tensor_reduce(
                    out=n2t[:],
                    in_=mzt[:].rearrange("p (g kq) -> p g kq", kq=PS),
                    op=ALU.add, axis=AX.X)
                if t > 0:
                    n2s = small.tile([128, 8], F32, tag=f"n2s{H}")
                    nc.vector.tensor_mul(n2s[:], n2t[:], r2[H][:])
                    n2t = n2s
                n2[H] = n2t

            # ---- squash factor fr = recd * n2/(1+n2)/sqrt(n2+eps)
            fr = {}
            for H in range(2):
                tln = small.tile([128, 8], F32, tag=f"tln{H}")
                nc.scalar.activation(tln[:], n2[H][:], AF.Ln, bias=epsc[:])
                rr = small.tile([128, 8], F32, tag=f"rr{H}")
                nc.scalar.activation(rr[:], tln[:], AF.Exp, scale=-0.5)
                dd = small.tile([128, 8], F32, tag=f"dd{H}")
                nc.vector.tensor_scalar_add(dd[:], n2[H][:], 1.0)
                rec = small.tile([128, 8], F32, tag=f"rec{H}")
                nc.vector.reciprocal(rec[:], dd[:])
                ff = small.tile([128, 8], F32, tag=f"ff{H}")
                nc.vector.tensor_mul(ff[:], n2[H][:], rec[:])
                ff2 = small.tile([128, 8], F32, tag=f"ff2{H}")
                nc.vector.tensor_mul(ff2[:], ff[:], rr[:])
                if t > 0:
                    frt = small.tile([128, 8], F32, tag=f"fr{H}")
                    nc.vector.tensor_mul(frt[:], ff2[:], recd[H][:])
                    fr[H] = frt
                else:
                    fr[H] = ff2

            if t == 2:
                # ---- output Pout = fr * S'
                for H in range(2):
                    outsb = state.tile([128, 128], F32, tag=f"outsb{H}")
                    frbc = fr[H][:].unsqueeze(2).broadcast_to((128, 8, PS))
                    nc.vector.tensor_tensor(
                        outsb[:].rearrange("p (g kq) -> p g kq", kq=PS),
                        s[H][:].rearrange("p (g kq) -> p g kq", kq=PS),
                        frbc, op=ALU.mult)
                    dst_o = o32[H].rearrange("(jj aL) b kq -> (aL b) jj kq",
                                             jj=J)
                    nc.sync.dma_start(
                        out=dst_o,
                        in_=outsb[:].rearrange("p (g kq) -> p g kq", kq=PS))
                continue

            # ---- U = fr*Z' ; Ubar += U; transpose + stage + stripe
            # copies per half so bi0/1's L matmuls never wait on half 1
            uthp = mbps_pool.tile([128, 512], F16, tag="uthp")
            ust = work.tile([128, 512], F16, tag="ust")
            for H in range(2):
                ubv = UB[t][H][:].rearrange("p (g kq2) -> p g kq2",
                                            kq2=32)[:, :, 0:PS]
                zv = z[H][:].rearrange("p (g kq) -> p g kq", kq=PS)
                frbc = fr[H][:].unsqueeze(2).broadcast_to((128, 8, PS))
                if t == 0:
                    nc.vector.tensor_tensor(ubv, zv, frbc, op=ALU.mult)
                else:
                    u16 = work.tile([128, 128], F16, tag=f"u16{H}")
                    u16v = u16[:].rearrange("p (g kq) -> p g kq", kq=PS)
                    nc.vector.tensor_tensor(u16v, zv, frbc, op=ALU.mult)
                    ub0v = UB[0][H][:].rearrange("p (g kq2) -> p g kq2",
                                                 kq2=32)[:, :, 0:PS]
                    nc.vector.tensor_tensor(ubv, ub0v, u16v, op=ALU.add)
                for half in range(2):
                    bi = H * 2 + half
                    nc.tensor.transpose(
                        uthp[:, bi * 128:(bi + 1) * 128],
                        UB[t][H][:, half * 128:(half + 1) * 128], IDEN[:])
                nc.scalar.activation(ust[:, H * 256:(H + 1) * 256],
                                     uthp[:, H * 256:(H + 1) * 256],
                                     AF.Identity)
                for g4 in range(4):
                    srcu = ust[g4 * 32:(g4 + 1) * 32,
                               H * 256:(H + 1) * 256]
                    dstu = UTS[g4][H][g4 * 32:(g4 + 1) * 32, :]
                    nc.vector.tensor_copy(dstu, srcu)

            if _STOP == f"t{t}ub":
                _dump32(UB[t][0][:, 0:256])
                return
            if _STOP == f"t{t}uts":
                _dump32(UTS[0][0][:, 0:256])
                return
            if _STOP == f"t{t}uthp":
                _dump32(uthp[:, 0:256])
                return

            # ---- L matmuls: L[h, (aL b)] = sum_(g4 kq32) MTD32 * UTS
            lp_tiles = {}
            for bi in range(4):
                lp = lps_pool.tile([128, 1024], F32, tag="lps")
                lp_tiles[bi] = lp
                for gi in range(4):
                    for ch in range(2):
                        for aL in range(4):
                            lhsT = MTD32[:, ((bi * 2 + ch) * 4 + aL) * 128:
                                         ((bi * 2 + ch) * 4 + aL + 1) * 128]
                            rhs = UTS[gi][bi // 2][
                                :, ((bi % 2) * 4 + aL) * 32:
                                ((bi % 2) * 4 + aL + 1) * 32]
                            nc.tensor.matmul(
                                lp[:, ch * 512 + gi * 128 + aL * 32:
                                   ch * 512 + gi * 128 + (aL + 1) * 32],
                                lhsT, rhs, start=True, stop=True)
                # exp of this lp right away (off the critical path)
                el = elp.tile([128, 1024], BF16, tag="el")
                nc.scalar.activation(el[:], lp[:], AF.Exp)
                el_tiles[bi] = el
            if _STOP == f"t{t}lp":
                _dump32(lp_tiles[0][:, 0:256])
                return


def _build_kernel():
    nc = bacc.Bacc("TRN2", target_bir_lowering=False, debug=False,
                   num_devices=NCORES)
    mtd = nc.dram_tensor("mtd", [128, 32 * 128], F16,
                         kind="ExternalInput").ap()
    xh16 = nc.dram_tensor("xh16", [NL, HW, A * PS], BF16,
                          kind="ExternalInput").ap()
    xl16 = nc.dram_tensor("xl16", [NL, HW, A * PS], BF16,
                          kind="ExternalInput").ap()
    wga = nc.dram_tensor("wga", [128, G * 16], F32, kind="ExternalInput").ap()
    wws = nc.dram_tensor("wws", [128, G * 16], F32, kind="ExternalInput").ap()
    iden = nc.dram_tensor("iden", [128, 128], F16, kind="ExternalInput").ap()
    o32 = nc.dram_tensor("o32", [NL, A, B, PS], F32,
                         kind="ExternalOutput").ap()

    with tile.TileContext(nc) as tc:
        _emit(tc, mtd, xh16, xl16, wga, wws, iden, o32)

    nc.compile()
    return nc


# ---------------------------------------------------------------- host side
def _host_weights(weights):
    W = np.asarray(weights, np.float32)                # (A, B, P, P)
    Gm = np.einsum("abpk,abpl->abkl", W, W)            # (A, B, 4, 4)
    Gsw = np.swapaxes(Gm, 2, 3)                        # Gsw[a,b,kp,k]=Gm[k,kp]
    Wsw = np.swapaxes(W, 2, 3)                         # Wsw[a,b,k,pp]=W[pp,k]

    wga = np.zeros((4, B, G, 4, 4), np.float32)        # (aL,b,g,kp,k)
    wws = np.zeros((4, B, G, 4, 4), np.float32)        # (aL,b,g,k,pp)
    for g in range(G):
        j = g % J                                      # g = nl*8 + j
        wga[:, :, g] = Gsw[4 * j:4 * j + 4]
        wws[:, :, g] = Wsw[4 * j:4 * j + 4]
    wga = wga.reshape(4 * B, G * 16)
    wws = wws.reshape(4 * B, G * 16)
    return wga.astype(np.float32), wws.astype(np.float32)


def _host_prep(x, weights):
    xr = np.asarray(x, np.float32).reshape(BATCH, HW, A, PS)
    wga, wws = _host_weights(weights)
    iden = np.eye(128, dtype=np.float16)

    in_maps = []
    for c in range(NCORES):
        xc = xr[c * NL:(c + 1) * NL]                   # (NL, HW, A, PS)
        xh = xc.astype(ml_dtypes.bfloat16)
        xl = (xc - xh.astype(np.float32)).astype(ml_dtypes.bfloat16)
        # mtd[(g4 kq32), ((bi, ch, aL), h)] = xc[nl, ch*128+h, 4j+aL, kq]
        # for kq<16 (zeros at kq>=16), with g = bi*4 + g4 = nl*8 + j.
        xj = xc.reshape(NL, 2, 128, J, 4, PS)          # nl,ch,h,j,aL,kq
        mt = np.zeros((4, 32, 4, 2, 4, 128), np.float32)
        for bi in range(4):
            nl = bi // 2
            for g4 in range(4):
                j = (bi % 2) * 4 + g4
                # xj[nl, :, :, j] is (ch, h, aL, kq) -> (kq, ch, aL, h)
                mt[g4, 0:PS, bi] = xj[nl, :, :, j].transpose(3, 0, 2, 1)
        mtdc = mt.reshape(128, 4096).astype(np.float16)
        in_maps.append({
            "mtd": np.ascontiguousarray(mtdc),
            "xh16": np.ascontiguousarray(xh.reshape(NL, HW, A * PS)),
            "xl16": np.ascontiguousarray(xl.reshape(NL, HW, A * PS)),
            "wga": wga,
            "wws": wws,
            "iden": iden,
        })
    return in_maps


_NC_CACHE = {}


def kernel(x, weights):
    if "nc" not in _NC_CACHE:
        _NC_CACHE["nc"] = _build_kernel()
    nc = _NC_CACHE["nc"]
    in_maps = _host_prep(x, weights)
    res = run_bass_kernel_spmd(nc, in_maps, list(range(NCORES)))
    out = np.concatenate([res.results[c]["o32"] for c in range(NCORES)],
                         axis=0)
    return out.astype(np.float32)
